# revision 1
# baseline (speedup 1.0000x reference)
"""Trainium2 Bass kernel for a custom transformer block.

Sharding: 8 cores = 4 batches x 2 sequence halves. Each core computes the
full block (LN1 -> QKV -> windowed attention -> LN2 -> MLP -> residual) for
its 1024 query tokens; the KV window (last 1024 tokens of its batch) is
recomputed on both cores of a batch pair to avoid any collectives.

Layout strategy: layernorms run token-major (free-dim reductions), matmul
operands are kept dim-major via PE transposes whose PSUM evacuation also
applies the LN gain/bias (per-partition scale/bias on the Scalar engine).
Attention scores are computed transposed ([key, query]) so no softmax
transposes are needed; row sums come free from an extra ones-column on V
and the normalization folds into the attention-output evacuation. All
matmuls run as float32r.
"""
import sys
import os

if "/opt/trn_rl_repo" not in sys.path:
    sys.path.insert(0, "/opt/trn_rl_repo")

import numpy as np
import ml_dtypes

B, S, D = 4, 2048, 1024
N_HEAD = 16
D_HEAD = 64
WINDOW = 1024
D_FF = 4096
EPS = 1e-5
ISD = float(1.0 / np.sqrt(D))  # 1/32
MASKVAL = -80.0   # exp(-80) = 1.8e-35: exact on ACT LUT, keeps fully-masked
KEEPVAL = 3e38    # rows uniform like the reference's -1e10 + softmax
P = 128

_CACHE = {}


def _build_program():
    import concourse.bacc as bacc
    import concourse.mybir as mybir
    from concourse.tile import TileContext
    from concourse.masks import make_identity

    F32 = mybir.dt.float32
    F32R = mybir.dt.float32r
    BF16 = mybir.dt.bfloat16
    AF = mybir.ActivationFunctionType
    ALU = mybir.AluOpType
    AX = mybir.AxisListType

    nc = bacc.Bacc("TRN2", target_bir_lowering=False, debug=False,
                   num_devices=8)

    xin_d = nc.dram_tensor("xin", [2 * WINDOW, D], F32, kind="ExternalInput")
    maskT_d = nc.dram_tensor("maskT", [WINDOW, WINDOW], BF16,
                             kind="ExternalInput")
    wq_d = nc.dram_tensor("wq", [D, D], F32R, kind="ExternalInput")
    wkv_d = nc.dram_tensor("wkv", [D, 2 * D], F32R, kind="ExternalInput")
    w1_d = nc.dram_tensor("w1", [D, D_FF], F32R, kind="ExternalInput")
    w2_d = nc.dram_tensor("w2", [D_FF, D], F32R, kind="ExternalInput")
    bqs_d = nc.dram_tensor("bqs", [P, 8], F32, kind="ExternalInput")
    bkvk_d = nc.dram_tensor("bkvk", [P, 8], F32, kind="ExternalInput")
    bkvvb_d = nc.dram_tensor("bkvvb", [P, D], F32, kind="ExternalInput")
    g1dm_d = nc.dram_tensor("g1dm", [P, 8], F32, kind="ExternalInput")
    b1dm_d = nc.dram_tensor("b1dm", [P, 8], F32, kind="ExternalInput")
    g2dm_d = nc.dram_tensor("g2dm", [P, 8], F32, kind="ExternalInput")
    b2dm_d = nc.dram_tensor("b2dm", [P, 8], F32, kind="ExternalInput")
    b1s_d = nc.dram_tensor("b1s", [P, 32], F32, kind="ExternalInput")
    b2s_d = nc.dram_tensor("b2s", [P, 8], F32, kind="ExternalInput")
    xinT_d = nc.dram_tensor("xinT", [D, WINDOW], F32, kind="ExternalInput")
    y_d = nc.dram_tensor("y", [D, WINDOW], F32, kind="ExternalOutput")

    with TileContext(nc) as tc:
        cpool = tc.alloc_tile_pool(name="const", bufs=1, side="left")
        ident = cpool.tile([P, P], F32)
        make_identity(nc, ident[:])
        smallc = cpool.tile([P, 104], F32)
        bqs = smallc[:, 0:8]
        bkvk = smallc[:, 8:16]
        b1s = smallc[:, 16:48]
        b2s = smallc[:, 48:56]
        onesc = smallc[:, 56:72]
        g1dm = smallc[:, 72:80]
        b1dm = smallc[:, 80:88]
        g2dm = smallc[:, 88:96]
        b2dm = smallc[:, 96:104]
        nc.vector.memset(onesc, 1.0)
        nc.sync.dma_start(bqs, bqs_d[:])
        nc.sync.dma_start(bkvk, bkvk_d[:])
        nc.sync.dma_start(b1s, b1s_d[:])
        nc.sync.dma_start(b2s, b2s_d[:])
        nc.sync.dma_start(g1dm, g1dm_d[:])
        nc.sync.dma_start(b1dm, b1dm_d[:])
        nc.sync.dma_start(g2dm, g2dm_d[:])
        nc.sync.dma_start(b2dm, b2dm_d[:])

        # ---------------- Phase B: LN1 + transpose to dim-major ------------
        # z = (x - mu) * rstd in token-major; gain/bias applied per-dim
        # during the transposed PSUM evacuation on ScalarE.
        zTp = tc.alloc_tile_pool(name="zT", bufs=1, side="left")
        zqT = zTp.tile([P, 8, WINDOW], F32R)
        zwT = zTp.tile([P, 8, WINDOW], F32R)
        xz = tc.alloc_tile_pool(name="xz", bufs=3, side="left")
        psB = tc.alloc_tile_pool(name="psB", bufs=3, space="PSUM")

        def ln1_tile(t):
            xt = xz.tile([P, D], F32, tag="x")
            nc.sync.dma_start(xt[:], xin_d[t * P:(t + 1) * P, :])
            st = xz.tile([P, 8], F32, tag="stats")
            musum, mu, vsum = st[:, 0:1], st[:, 1:2], st[:, 2:3]
            veps, sdv, rstd = st[:, 4:5], st[:, 5:6], st[:, 6:7]
            nc.vector.reduce_sum(musum, xt[:], axis=AX.X)
            nc.vector.tensor_scalar_mul(mu, musum, 1.0 / D)
            z = xz.tile([P, D], F32, tag="z")
            # scratch into z; vsum = sum((x - mu) * x) = D * var
            nc.vector.scalar_tensor_tensor(
                z[:], xt[:], mu, xt[:],
                op0=ALU.subtract, op1=ALU.mult, accum_out=vsum)
            nc.vector.tensor_scalar(veps, vsum, 1.0 / D, EPS,
                                    op0=ALU.mult, op1=ALU.add)
            nc.scalar.sqrt(sdv, veps)
            nc.vector.reciprocal(rstd, sdv)
            nc.vector.tensor_scalar(z[:], xt[:], mu, rstd,
                                    op0=ALU.subtract, op1=ALU.mult)
            dst = zqT if t < 8 else zwT
            col = (t % 8) * P
            for c in range(8):
                tp = psB.tile([P, P], F32, tag="tpB")
                nc.tensor.transpose(tp[:], z[:, c * P:(c + 1) * P], ident[:])
                nc.scalar.activation(dst[:, c, col:col + P], tp[:],
                                     AF.Identity, bias=b1dm[:, c:c + 1],
                                     scale=g1dm[:, c:c + 1])

        # interleave: LN(query half) -> Q proj -> LN(window half) -> V -> K
        # so Q matmuls fill PE idle during LN and attention starts during K.
        for t in range(8):
            ln1_tile(t)

        qkvp = tc.alloc_tile_pool(name="qkv", bufs=1, side="right")
        qT = qkvp.tile([P, 8, WINDOW], F32R)      # q/sqrt(D), dim-major
        kT = qkvp.tile([P, 8, WINDOW], F32R)      # k, dim-major
        V = qkvp.tile([P, 8, N_HEAD * 65], F32R)  # token-major + ones col

        wst = tc.alloc_tile_pool(name="wst", bufs=1, side="left")
        psC = tc.alloc_tile_pool(name="psC", bufs=4, space="PSUM")

        # Q: weights stationary -> qT dim-major, scaled by 1/32
        for wh in range(2):
            wqr = wst.tile([P, 8, 512], F32R, tag="wkres")
            for kc in range(8):
                nc.sync.dma_start(
                    wqr[:, kc, :],
                    wq_d[kc * P:(kc + 1) * P, wh * 512:(wh + 1) * 512])
            for co in range(wh * 4, wh * 4 + 4):
                for qh in range(2):
                    pp = psC.tile([P, 512], F32, tag="proj")
                    for kc in range(8):
                        nc.tensor.matmul(
                            pp[:], wqr[:, kc, (co % 4) * P:(co % 4 + 1) * P],
                            zqT[:, kc, qh * 512:(qh + 1) * 512],
                            start=(kc == 0), stop=(kc == 7))
                    nc.scalar.activation(
                        qT[:, co, qh * 512:(qh + 1) * 512], pp[:],
                        AF.Identity, bias=bqs[:, co:co + 1], scale=ISD)

        for t in range(8, 16):
            ln1_tile(t)

        # V: activations stationary -> token-major, bias added via bcast tile
        bkvvb = wst.tile([P, D], F32, tag="bkvvb")
        nc.sync.dma_start(bkvvb[:], bkvvb_d[:])
        for vh in range(2):
            wvr = wst.tile([P, 8, 512], F32R, tag="wkres")
            for kc in range(8):
                nc.sync.dma_start(
                    wvr[:, kc, :],
                    wkv_d[kc * P:(kc + 1) * P,
                          D + vh * 512:D + (vh + 1) * 512])
            for tt in range(8):
                pp = psC.tile([P, 512], F32, tag="proj")
                for kc in range(8):
                    nc.tensor.matmul(
                        pp[:], zwT[:, kc, tt * P:(tt + 1) * P],
                        wvr[:, kc, :],
                        start=(kc == 0), stop=(kc == 7))
                vdst = V[:, tt, :].rearrange("p (h n) -> p h n", n=65)[
                    :, vh * 8:(vh + 1) * 8, 0:64]
                nc.vector.scalar_tensor_tensor(
                    vdst, pp[:].rearrange("p (h n) -> p h n", n=64), 0.0,
                    bkvvb[:, vh * 512:(vh + 1) * 512].rearrange(
                        "p (h n) -> p h n", n=64),
                    op0=ALU.add, op1=ALU.add)
        for tt in range(8):
            nc.scalar.copy(
                V[:, tt, :].rearrange("p (h n) -> p h n", n=65)[:, :, 64:65],
                onesc.rearrange("p (h n) -> p h n", n=1))
        # K: weights stationary -> kT dim-major (last so D overlaps it)
        for wh in range(2):
            wkr = wst.tile([P, 8, 512], F32R, tag="wkres")
            for kc in range(8):
                nc.sync.dma_start(
                    wkr[:, kc, :],
                    wkv_d[kc * P:(kc + 1) * P, wh * 512:(wh + 1) * 512])
            for co in range(wh * 4, wh * 4 + 4):
                for qh in range(2):
                    pp = psC.tile([P, 512], F32, tag="proj")
                    for kc in range(8):
                        nc.tensor.matmul(
                            pp[:], wkr[:, kc, (co % 4) * P:(co % 4 + 1) * P],
                            zwT[:, kc, qh * 512:(qh + 1) * 512],
                            start=(kc == 0), stop=(kc == 7))
                    nc.scalar.activation(
                        kT[:, co, qh * 512:(qh + 1) * 512], pp[:],
                        AF.Identity, bias=bkvk[:, co:co + 1], scale=1.0)

        psC.release()
        psB.release()
        wst.release()
        xz.release()
        zTp.release()

        # ---------------- Phase D: attention --------------------------------
        attnp = tc.alloc_tile_pool(name="attn", bufs=1, side="left")
        attn = attnp.tile([P, 8, D], F32)          # normalized attn out
        asum = attnp.tile([P, 8, N_HEAD], F32)     # per-head row sums
        rinva = attnp.tile([P, 8, N_HEAD], F32)    # per-head 1/rowsum

        mkp = tc.alloc_tile_pool(name="mk", bufs=1, side="left")
        maskT = mkp.tile([P, 8, WINDOW], BF16)
        nc.sync.dma_start(maskT[:], maskT_d.rearrange("(c p) n -> p c n", p=P))
        sbD = tc.alloc_tile_pool(name="sbD", bufs=2, side="left")
        ssbp = tc.alloc_tile_pool(name="ssbp", bufs=4, side="left")
        ptp = tc.alloc_tile_pool(name="ptp", bufs=9, side="right")
        psDs = tc.alloc_tile_pool(name="psDs", bufs=5, space="PSUM")
        psDa = tc.alloc_tile_pool(name="psDa", bufs=2, space="PSUM")
        psDt = tc.alloc_tile_pool(name="psDt", bufs=1, space="PSUM")

        for hp in range(N_HEAD // 2):
            pair = (2 * hp, 2 * hp + 1)
            # scores + mask + exp; consecutive matmuls alternate PE row
            # groups (partitions 0-63 vs 64-127) so LDWEIGHTS pulls ahead
            pts = {h: [] for h in pair}
            for kc in range(8):
                ssbs = {}
                for h in pair:
                    ptile = ptp.tile([P, 1024], F32R, tag="pt")
                    pts[h].append(ptile)
                    stile = ssbp.tile([P, 1024], F32, tag="ssb")
                    ssbs[h] = stile
                for qh in range(2):
                    for h in pair:
                        po, ch = (h % 2) * 64, h // 2
                        sps = psDs.tile([P, 512], F32, tag="s")
                        nc.tensor.matmul(
                            sps[:],
                            kT[po:po + 64, ch, kc * P:(kc + 1) * P],
                            qT[po:po + 64, ch, qh * 512:(qh + 1) * 512],
                            start=True, stop=True)
                        nc.vector.tensor_tensor(
                            ssbs[h][:, qh * 512:(qh + 1) * 512], sps[:],
                            maskT[:, kc, qh * 512:(qh + 1) * 512], op=ALU.min)
                for h in pair:
                    nc.scalar.activation(pts[h][kc][:], ssbs[h][:], AF.Exp)
            for h in pair:
                oa = sbD.tile([65, 1024], F32, tag="oa")
                for qh in range(2):
                    avp = psDa.tile([65, 512], F32, tag="av")
                    for kc in range(8):
                        nc.tensor.matmul(
                            avp[:], V[:, kc, h * 65:(h + 1) * 65],
                            pts[h][kc][:, qh * 512:(qh + 1) * 512],
                            start=(kc == 0), stop=(kc == 7))
                    nc.scalar.copy(oa[:, qh * 512:(qh + 1) * 512], avp[:])
                for t in range(8):
                    tp = psDt.tile([P, 65], F32, tag="tp65")
                    nc.tensor.transpose(tp[:], oa[:, t * P:(t + 1) * P],
                                        ident[0:65, 0:65])
                    rinv = rinva[:, t, h:h + 1]
                    nc.vector.reciprocal(rinv, tp[:, 64:65])
                    nc.scalar.activation(
                        attn[:, t, h * 64:(h + 1) * 64], tp[:, 0:64],
                        AF.Copy, scale=rinv, accum_out=asum[:, t, h:h + 1])

        psDt.release()
        psDa.release()
        psDs.release()
        ptp.release()
        ssbp.release()
        sbD.release()
        mkp.release()
        qkvp.release()

        # ---------------- Phase E: LN2 + transpose ---------------------------
        z2Tp = tc.alloc_tile_pool(name="z2T", bufs=1, side="right")
        z2T = z2Tp.tile([P, 8, WINDOW], F32R)
        xz2 = tc.alloc_tile_pool(name="xz2", bufs=3, side="left")
        psE = tc.alloc_tile_pool(name="psE", bufs=3, space="PSUM")

        for t in range(8):
            at = attn[:, t, :]
            st = xz2.tile([P, 8], F32, tag="stats2")
            musum, mu, vsum = st[:, 0:1], st[:, 1:2], st[:, 2:3]
            veps, sdv, rstd = st[:, 4:5], st[:, 5:6], st[:, 6:7]
            nc.vector.reduce_sum(musum, asum[:, t, :], axis=AX.X)
            nc.vector.tensor_scalar_mul(mu, musum, 1.0 / D)
            z = xz2.tile([P, D], F32, tag="zE")
            nc.vector.scalar_tensor_tensor(
                z[:], at, mu, at,
                op0=ALU.subtract, op1=ALU.mult, accum_out=vsum)
            nc.vector.tensor_scalar(veps, vsum, 1.0 / D, EPS,
                                    op0=ALU.mult, op1=ALU.add)
            nc.scalar.sqrt(sdv, veps)
            nc.vector.reciprocal(rstd, sdv)
            nc.vector.tensor_scalar(z[:], at, mu, rstd,
                                    op0=ALU.subtract, op1=ALU.mult)
            for c in range(8):
                tp = psE.tile([P, P], F32, tag="tpE")
                nc.tensor.transpose(tp[:], z[:, c * P:(c + 1) * P], ident[:])
                nc.scalar.activation(z2T[:, c, t * P:(t + 1) * P], tp[:],
                                     AF.Identity, bias=b2dm[:, c:c + 1],
                                     scale=g2dm[:, c:c + 1])

        psE.release()
        xz2.release()
        attnp.release()

        # ---------------- Phase F: MLP ---------------------------------------
        h2p = tc.alloc_tile_pool(name="h2acc", bufs=1, side="left")
        h2acc = h2p.tile([P, 8, WINDOW], F32)
        xinTp = tc.alloc_tile_pool(name="xinT", bufs=1, side="left")
        xinT = xinTp.tile([P, 8, WINDOW], F32)
        nc.sync.dma_start(xinT[:], xinT_d.rearrange("(c p) n -> p c n", p=P))
        wf1 = tc.alloc_tile_pool(name="wf1", bufs=1, side="right")
        wf2 = tc.alloc_tile_pool(name="wf2", bufs=1, side="right")
        h1p = tc.alloc_tile_pool(name="h1p", bufs=1, side="left")
        psF1 = tc.alloc_tile_pool(name="psF1", bufs=4, space="PSUM")
        psF2 = tc.alloc_tile_pool(name="psF2", bufs=4, space="PSUM")

        for sc in range(4):
            w1r = wf1.tile([P, 8, 1024], F32R, tag="w1r")
            for kc in range(8):
                nc.sync.dma_start(
                    w1r[:, kc, :],
                    w1_d[kc * P:(kc + 1) * P, sc * 1024:(sc + 1) * 1024])
            h1 = h1p.tile([P, 8, WINDOW], F32R, tag="h1")
            for ft in range(8):
                for qh in range(2):
                    hp = psF1.tile([P, 512], F32, tag="h1ps")
                    for kc in range(8):
                        nc.tensor.matmul(
                            hp[:], w1r[:, kc, ft * P:(ft + 1) * P],
                            z2T[:, kc, qh * 512:(qh + 1) * 512],
                            start=(kc == 0), stop=(kc == 7))
                    nc.scalar.activation(
                        h1[:, ft, qh * 512:(qh + 1) * 512], hp[:], AF.Silu,
                        bias=b1s[:, sc * 8 + ft:sc * 8 + ft + 1], scale=1.0)
            w2r = wf2.tile([P, 8, 1024], F32R, tag="w2r")
            for kc in range(8):
                nc.sync.dma_start(
                    w2r[:, kc, :],
                    w2_d[(sc * 8 + kc) * P:(sc * 8 + kc + 1) * P, :])
            for co in range(8):
                for qh in range(2):
                    hp2 = psF2.tile([P, 512], F32, tag="h2ps")
                    for kc in range(8):
                        nc.tensor.matmul(
                            hp2[:], w2r[:, kc, co * P:(co + 1) * P],
                            h1[:, kc, qh * 512:(qh + 1) * 512],
                            start=(kc == 0), stop=(kc == 7))
                    dstp = h2acc[:, co, qh * 512:(qh + 1) * 512]
                    if sc == 0:
                        nc.vector.tensor_copy(dstp, hp2[:])
                    elif sc < 3:
                        nc.vector.tensor_tensor(dstp, hp2[:], dstp,
                                                op=ALU.add)
                    else:
                        nc.vector.scalar_tensor_tensor(
                            dstp, hp2[:], b2s[:, co:co + 1], dstp,
                            op0=ALU.add, op1=ALU.add)

        psF2.release()
        psF1.release()
        h1p.release()
        wf2.release()
        wf1.release()
        z2Tp.release()

        # tail: residual add in dim-major (host supplies x^T and
        # transposes y back), no PE transposes needed
        tailp = tc.alloc_tile_pool(name="tail", bufs=3, side="left")
        for co in range(8):
            y = tailp.tile([P, WINDOW], F32, tag="y")
            nc.vector.tensor_tensor(y[:], h2acc[:, co, :], xinT[:, co, :],
                                    op=ALU.add)
            nc.sync.dma_start(y_d[co * P:(co + 1) * P, :], y[:])
        tailp.release()
        xinTp.release()
        h2p.release()
        cpool.release()

    nc.compile()
    return nc


def _prep_inputs(inputs):
    x = np.ascontiguousarray(np.asarray(inputs["x"], dtype=np.float32))
    kpm = np.asarray(inputs["key_pad_mask"]).astype(bool)
    wq = np.ascontiguousarray(np.asarray(inputs["wq"], dtype=np.float32))
    wkv = np.ascontiguousarray(np.asarray(inputs["wkv"], dtype=np.float32))
    w1 = np.ascontiguousarray(np.asarray(inputs["w1"], dtype=np.float32))
    w2 = np.ascontiguousarray(np.asarray(inputs["w2"], dtype=np.float32))
    bq = np.asarray(inputs["bq"], dtype=np.float32)
    bkv = np.asarray(inputs["bkv"], dtype=np.float32)
    b1 = np.asarray(inputs["b1"], dtype=np.float32)
    b2 = np.asarray(inputs["b2"], dtype=np.float32)
    ln1_g = np.asarray(inputs["ln1_g"], dtype=np.float32)
    ln1_b = np.asarray(inputs["ln1_b"], dtype=np.float32)
    ln2_g = np.asarray(inputs["ln2_g"], dtype=np.float32)
    ln2_b = np.asarray(inputs["ln2_b"], dtype=np.float32)

    def dm(v):  # [D] -> [P, 8] dim-major chunk layout
        return np.ascontiguousarray(v.reshape(8, P).T)

    shared = {
        "wq": wq,
        "wkv": wkv,
        "w1": w1,
        "w2": w2,
        "bqs": np.ascontiguousarray((bq * ISD).reshape(8, P).T),
        "bkvk": dm(bkv[0:D]),
        "bkvvb": np.ascontiguousarray(
            np.broadcast_to(bkv[D:2 * D], (P, D)).astype(np.float32)),
        "g1dm": dm(ln1_g),
        "b1dm": dm(ln1_b),
        "g2dm": dm(ln2_g),
        "b2dm": dm(ln2_b),
        "b1s": np.ascontiguousarray(b1.reshape(32, P).T),
        "b2s": dm(b2),
    }

    j = np.arange(WINDOW)[:, None]   # key index within window (row)
    i = np.arange(WINDOW)[None, :]   # local query index (col)
    in_maps = []
    for core in range(8):
        b, h = core // 2, core % 2
        xq = x[b, h * WINDOW:(h + 1) * WINDOW]
        xw = x[b, S - WINDOW:S]
        masked = (j > h * WINDOW + i) | kpm[b, S - WINDOW:S][:, None]
        maskT = np.where(masked, np.float32(MASKVAL),
                         np.float32(KEEPVAL)).astype(ml_dtypes.bfloat16)
        m = dict(shared)
        m["xin"] = np.ascontiguousarray(np.concatenate([xq, xw], axis=0))
        m["xinT"] = np.ascontiguousarray(xq.T)
        m["maskT"] = np.ascontiguousarray(maskT)
        in_maps.append(m)
    return in_maps


def kernel(**inputs):
    from concourse.bass_utils import run_bass_kernel_spmd

    if "nc" not in _CACHE:
        _CACHE["nc"] = _build_program()
    nc = _CACHE["nc"]

    in_maps = _prep_inputs(inputs)
    trace = os.environ.get("KERNEL_TRACE", "0") == "1"
    res = run_bass_kernel_spmd(nc, in_maps, core_ids=list(range(8)),
                               trace=trace)
    if res.exec_time_ns is not None:
        print(f"HW exec time: {res.exec_time_ns} ns")
        _CACHE["exec_time_ns"] = res.exec_time_ns
    out = np.empty((B, S, D), dtype=np.float32)
    for core in range(8):
        b, h = core // 2, core % 2
        out[b, h * WINDOW:(h + 1) * WINDOW] = res.results[core]["y"].T
    return out



# revision 5
# speedup vs baseline: 1.1266x; 1.1266x over previous
"""Trainium2 Bass kernel for a custom transformer block.

Sharding: 8 cores = 4 batches x 2 interleaved query-chunk sets. Core (b, h)
owns query chunks {2s+h : s in 0..8} (128 tokens each) of batch b; the KV
window (last 1024 tokens) is recomputed on both cores of a batch pair. The
stride-2 interleave balances the causal-triangular attention work across the
pair and lets the score matmuls skip fully-masked key/query blocks: the
score strip for key chunk kc only covers queries from slot ceil((kc-1)/2).

All matmul operands are bf16 (fp32 PSUM accumulation); LN stats, softmax
denominators, the MLP output accumulator and the residual stay fp32.
Padding masks are folded into the exp evacuation as a per-key ACT bias
(-80 * pad); only the first 128-query block of each key strip needs a 2D
min-mask (causal diagonal / full block), using two per-core constant tiles.
Attention scores are computed transposed ([key, query]); row sums come from
an extra ones-column on V and the normalization folds into the attention
output evacuation.
"""
import sys
import os

if "/opt/trn_rl_repo" not in sys.path:
    sys.path.insert(0, "/opt/trn_rl_repo")

import numpy as np
import ml_dtypes

B, S, D = 4, 2048, 1024
N_HEAD = 16
D_HEAD = 64
WINDOW = 1024
D_FF = 4096
EPS = 1e-5
ISD = float(1.0 / np.sqrt(D))  # 1/32
MASKVAL = -80.0   # exp(-80) = 1.8e-35: effectively zero, no underflow->NaN
KEEPVAL = 3e38
P = 128

# first live query slot for key chunk kc (strip start = 128*S_MIN[kc]);
# slot s holds query chunk 2s+h, live when kc <= 2s+h -> s >= ceil((kc-1)/2)
S_MIN = [kc // 2 for kc in range(8)]  # == ceil((kc-1)/2): [0,0,1,1,2,2,3,3]

_CACHE = {}


def _build_program():
    import concourse.bacc as bacc
    import concourse.mybir as mybir
    from concourse.tile import TileContext
    from concourse.masks import make_identity

    F32 = mybir.dt.float32
    BF16 = mybir.dt.bfloat16
    AF = mybir.ActivationFunctionType
    ALU = mybir.AluOpType
    AX = mybir.AxisListType

    nc = bacc.Bacc("TRN2", target_bir_lowering=False, debug=False,
                   num_devices=8)

    xin_d = nc.dram_tensor("xin", [2 * WINDOW, D], F32, kind="ExternalInput")
    wq_d = nc.dram_tensor("wq", [D, D], BF16, kind="ExternalInput")
    wkv_d = nc.dram_tensor("wkv", [D, 2 * D], BF16, kind="ExternalInput")
    w1_d = nc.dram_tensor("w1", [D, D_FF], BF16, kind="ExternalInput")
    w2_d = nc.dram_tensor("w2", [D_FF, D], BF16, kind="ExternalInput")
    bqs_d = nc.dram_tensor("bqs", [P, 8], F32, kind="ExternalInput")
    bkvk_d = nc.dram_tensor("bkvk", [P, 8], F32, kind="ExternalInput")
    bkvvb_d = nc.dram_tensor("bkvvb", [P, D], F32, kind="ExternalInput")
    g1dm_d = nc.dram_tensor("g1dm", [P, 8], F32, kind="ExternalInput")
    b1dm_d = nc.dram_tensor("b1dm", [P, 8], F32, kind="ExternalInput")
    g2dm_d = nc.dram_tensor("g2dm", [P, 8], F32, kind="ExternalInput")
    b2dm_d = nc.dram_tensor("b2dm", [P, 8], F32, kind="ExternalInput")
    b1s_d = nc.dram_tensor("b1s", [P, 32], F32, kind="ExternalInput")
    b2s_d = nc.dram_tensor("b2s", [P, 8], F32, kind="ExternalInput")
    padb_d = nc.dram_tensor("padb", [P, 8], F32, kind="ExternalInput")
    maskE_d = nc.dram_tensor("maskE", [P, P], BF16, kind="ExternalInput")
    maskO_d = nc.dram_tensor("maskO", [P, P], BF16, kind="ExternalInput")
    xinT_d = nc.dram_tensor("xinT", [D, WINDOW], F32, kind="ExternalInput")
    y_d = nc.dram_tensor("y", [D, WINDOW], F32, kind="ExternalOutput")

    with TileContext(nc) as tc:
        cpool = tc.alloc_tile_pool(name="const", bufs=1, side="left")
        identB = cpool.tile([P, P], BF16)
        make_identity(nc, identB[:])
        masks = cpool.tile([P, 2 * P], BF16)
        maskE = masks[:, 0:P]
        maskO = masks[:, P:2 * P]
        nc.sync.dma_start(maskE, maskE_d[:])
        nc.sync.dma_start(maskO, maskO_d[:])
        smallc = cpool.tile([P, 112], F32)
        bqs = smallc[:, 0:8]
        bkvk = smallc[:, 8:16]
        b1s = smallc[:, 16:48]
        b2s = smallc[:, 48:56]
        onesc = smallc[:, 56:72]
        g1dm = smallc[:, 72:80]
        b1dm = smallc[:, 80:88]
        g2dm = smallc[:, 88:96]
        b2dm = smallc[:, 96:104]
        padb = smallc[:, 104:112]
        nc.vector.memset(onesc, 1.0)
        nc.sync.dma_start(bqs, bqs_d[:])
        nc.sync.dma_start(bkvk, bkvk_d[:])
        nc.sync.dma_start(b1s, b1s_d[:])
        nc.sync.dma_start(b2s, b2s_d[:])
        nc.sync.dma_start(g1dm, g1dm_d[:])
        nc.sync.dma_start(b1dm, b1dm_d[:])
        nc.sync.dma_start(g2dm, g2dm_d[:])
        nc.sync.dma_start(b2dm, b2dm_d[:])
        nc.sync.dma_start(padb, padb_d[:])

        # ---------------- Phase B: LN1 + transpose to dim-major ------------
        # z = (x - mu) * rstd in token-major; gain/bias applied per-dim
        # during the transposed PSUM evacuation on ScalarE.
        zTp = tc.alloc_tile_pool(name="zT", bufs=1, side="left")
        zqT = zTp.tile([P, 8, WINDOW], BF16)
        zwT = zTp.tile([P, 8, WINDOW], BF16)
        xz = tc.alloc_tile_pool(name="xz", bufs=3, side="left")
        psB = tc.alloc_tile_pool(name="psB", bufs=3, space="PSUM")

        def ln1_tile(t):
            xt = xz.tile([P, D], F32, tag="x")
            nc.sync.dma_start(xt[:], xin_d[t * P:(t + 1) * P, :])
            st = xz.tile([P, 8], F32, tag="stats")
            musum, mu, vsum = st[:, 0:1], st[:, 1:2], st[:, 2:3]
            veps, sdv, rstd = st[:, 4:5], st[:, 5:6], st[:, 6:7]
            nc.vector.reduce_sum(musum, xt[:], axis=AX.X)
            nc.vector.tensor_scalar_mul(mu, musum, 1.0 / D)
            z = xz.tile([P, D], BF16, tag="z")
            zf = xz.tile([P, D], F32, tag="zf")
            # scratch into zf; vsum = sum((x - mu) * x) = D * var
            nc.vector.scalar_tensor_tensor(
                zf[:], xt[:], mu, xt[:],
                op0=ALU.subtract, op1=ALU.mult, accum_out=vsum)
            nc.vector.tensor_scalar(veps, vsum, 1.0 / D, EPS,
                                    op0=ALU.mult, op1=ALU.add)
            nc.scalar.sqrt(sdv, veps)
            nc.vector.reciprocal(rstd, sdv)
            nc.vector.tensor_scalar(z[:], xt[:], mu, rstd,
                                    op0=ALU.subtract, op1=ALU.mult)
            dst = zqT if t < 8 else zwT
            col = (t % 8) * P
            for c in range(8):
                tp = psB.tile([P, P], BF16, tag="tpB")
                nc.tensor.transpose(tp[:], z[:, c * P:(c + 1) * P], identB[:])
                nc.scalar.activation(dst[:, c, col:col + P], tp[:],
                                     AF.Identity, bias=b1dm[:, c:c + 1],
                                     scale=g1dm[:, c:c + 1])

        # interleave: LN(query half) -> Q proj -> LN(window half) -> V -> K
        # so Q matmuls fill PE idle during LN and attention starts during K.
        for t in range(8):
            ln1_tile(t)

        qkvp = tc.alloc_tile_pool(name="qkv", bufs=1, side="right")
        qT = qkvp.tile([P, 8, WINDOW], BF16)      # q/sqrt(D), dim-major
        kT = qkvp.tile([P, 8, WINDOW], BF16)      # k, dim-major
        V = qkvp.tile([P, 8, N_HEAD * 65], BF16)  # token-major + ones col

        wst = tc.alloc_tile_pool(name="wst", bufs=1, side="left")
        psC = tc.alloc_tile_pool(name="psC", bufs=4, space="PSUM")

        # Q: weights stationary -> qT dim-major, scaled by 1/32
        for wh in range(2):
            wqr = wst.tile([P, 8, 512], BF16, tag="wkres")
            for kc in range(8):
                nc.sync.dma_start(
                    wqr[:, kc, :],
                    wq_d[kc * P:(kc + 1) * P, wh * 512:(wh + 1) * 512])
            for co in range(wh * 4, wh * 4 + 4):
                for qh in range(2):
                    pp = psC.tile([P, 512], F32, tag="proj")
                    for kc in range(8):
                        nc.tensor.matmul(
                            pp[:], wqr[:, kc, (co % 4) * P:(co % 4 + 1) * P],
                            zqT[:, kc, qh * 512:(qh + 1) * 512],
                            start=(kc == 0), stop=(kc == 7))
                    nc.scalar.activation(
                        qT[:, co, qh * 512:(qh + 1) * 512], pp[:],
                        AF.Identity, bias=bqs[:, co:co + 1], scale=ISD)

        for t in range(8, 16):
            ln1_tile(t)

        # V: activations stationary -> token-major, bias added via bcast tile
        bkvvb = wst.tile([P, D], F32, tag="bkvvb")
        nc.sync.dma_start(bkvvb[:], bkvvb_d[:])
        for vh in range(2):
            wvr = wst.tile([P, 8, 512], BF16, tag="wkres")
            for kc in range(8):
                nc.sync.dma_start(
                    wvr[:, kc, :],
                    wkv_d[kc * P:(kc + 1) * P,
                          D + vh * 512:D + (vh + 1) * 512])
            for tt in range(8):
                pp = psC.tile([P, 512], F32, tag="proj")
                for kc in range(8):
                    nc.tensor.matmul(
                        pp[:], zwT[:, kc, tt * P:(tt + 1) * P],
                        wvr[:, kc, :],
                        start=(kc == 0), stop=(kc == 7))
                vdst = V[:, tt, :].rearrange("p (h n) -> p h n", n=65)[
                    :, vh * 8:(vh + 1) * 8, 0:64]
                nc.vector.scalar_tensor_tensor(
                    vdst, pp[:].rearrange("p (h n) -> p h n", n=64), 0.0,
                    bkvvb[:, vh * 512:(vh + 1) * 512].rearrange(
                        "p (h n) -> p h n", n=64),
                    op0=ALU.add, op1=ALU.add)
        for tt in range(8):
            nc.scalar.copy(
                V[:, tt, :].rearrange("p (h n) -> p h n", n=65)[:, :, 64:65],
                onesc.rearrange("p (h n) -> p h n", n=1))
        # K: weights stationary -> kT dim-major (last so D overlaps it)
        for wh in range(2):
            wkr = wst.tile([P, 8, 512], BF16, tag="wkres")
            for kc in range(8):
                nc.sync.dma_start(
                    wkr[:, kc, :],
                    wkv_d[kc * P:(kc + 1) * P, wh * 512:(wh + 1) * 512])
            for co in range(wh * 4, wh * 4 + 4):
                for qh in range(2):
                    pp = psC.tile([P, 512], F32, tag="proj")
                    for kc in range(8):
                        nc.tensor.matmul(
                            pp[:], wkr[:, kc, (co % 4) * P:(co % 4 + 1) * P],
                            zwT[:, kc, qh * 512:(qh + 1) * 512],
                            start=(kc == 0), stop=(kc == 7))
                    nc.scalar.activation(
                        kT[:, co, qh * 512:(qh + 1) * 512], pp[:],
                        AF.Identity, bias=bkvk[:, co:co + 1], scale=1.0)

        psC.release()
        psB.release()
        wst.release()
        xz.release()
        zTp.release()

        # ---------------- Phase D: attention --------------------------------
        attnp = tc.alloc_tile_pool(name="attn", bufs=1, side="left")
        attn = attnp.tile([P, 8, D], F32)          # normalized attn out
        asum = attnp.tile([P, 8, N_HEAD], F32)     # per-head row sums
        rinva = attnp.tile([P, 8, N_HEAD], F32)    # per-head 1/rowsum
        # ping-pong exp(score) tiles: [pair-head, kc, query]; the columns
        # before each strip start are never written -> zero them once
        ptsA = attnp.tile([P, 2, 8, WINDOW], BF16)
        ptsB = attnp.tile([P, 2, 8, WINDOW], BF16)
        for pts in (ptsA, ptsB):
            for h2 in range(2):
                for kc in range(2, 8):
                    z0 = S_MIN[kc] * P
                    nc.vector.memset(pts[:, h2, kc, 0:z0], 0.0)

        sbD = tc.alloc_tile_pool(name="sbD", bufs=2, side="left")
        psDs = tc.alloc_tile_pool(name="psDs", bufs=4, space="PSUM")
        psDa = tc.alloc_tile_pool(name="psDa", bufs=2, space="PSUM")
        psDt = tc.alloc_tile_pool(name="psDt", bufs=2, space="PSUM")

        for hp in range(N_HEAD // 2):
            pair = (2 * hp, 2 * hp + 1)
            pts = ptsA if hp % 2 == 0 else ptsB
            # scores + diag-block mask + exp; consecutive matmuls alternate
            # PE row groups (partitions 0-63 vs 64-127) so LDWEIGHTS pulls
            # ahead. Strip for key chunk kc covers queries [128*S_MIN, 1024).
            for kc in range(8):
                z0 = S_MIN[kc] * P
                L = WINDOW - z0
                pieces = [(0, 512), (512, L)]
                sts = {}
                for piece, (c0, c1) in enumerate(pieces):
                    for h in pair:
                        po, ch = (h % 2) * 64, h // 2
                        sp = psDs.tile([P, 512], F32, tag="s")
                        sts[(h, piece)] = sp
                        nc.tensor.matmul(
                            sp[:, 0:c1 - c0],
                            kT[po:po + 64, ch, kc * P:(kc + 1) * P],
                            qT[po:po + 64, ch, z0 + c0:z0 + c1],
                            start=True, stop=True)
                mk = maskE if kc % 2 == 0 else maskO
                for h in pair:
                    sp0 = sts[(h, 0)]
                    nc.vector.tensor_tensor(
                        sp0[:, 0:P], sp0[:, 0:P], mk, op=ALU.min)
                for piece, (c0, c1) in enumerate(pieces):
                    for h in pair:
                        nc.scalar.activation(
                            pts[:, h % 2, kc, z0 + c0:z0 + c1],
                            sts[(h, piece)][:, 0:c1 - c0],
                            AF.Exp, bias=padb[:, kc:kc + 1])
            for h in pair:
                oa = sbD.tile([65, 1024], BF16, tag="oa")
                for qh in range(2):
                    avp = psDa.tile([65, 512], F32, tag="av")
                    for kc in range(8):
                        nc.tensor.matmul(
                            avp[:], V[:, kc, h * 65:(h + 1) * 65],
                            pts[:, h % 2, kc, qh * 512:(qh + 1) * 512],
                            start=(kc == 0), stop=(kc == 7))
                    nc.vector.tensor_copy(oa[:, qh * 512:(qh + 1) * 512],
                                          avp[:])
                for t in range(8):
                    tp = psDt.tile([P, 65], BF16, tag="tp65")
                    nc.tensor.transpose(tp[:], oa[:, t * P:(t + 1) * P],
                                        identB[0:65, 0:65])
                    rinv = rinva[:, t, h:h + 1]
                    nc.vector.reciprocal(rinv, tp[:, 64:65])
                    nc.scalar.activation(
                        attn[:, t, h * 64:(h + 1) * 64], tp[:, 0:64],
                        AF.Copy, scale=rinv, accum_out=asum[:, t, h:h + 1])

        psDt.release()
        psDa.release()
        psDs.release()
        sbD.release()
        qkvp.release()

        # ---------------- Phase E: LN2 + transpose ---------------------------
        z2Tp = tc.alloc_tile_pool(name="z2T", bufs=1, side="right")
        z2T = z2Tp.tile([P, 8, WINDOW], BF16)
        xz2 = tc.alloc_tile_pool(name="xz2", bufs=3, side="left")
        psE = tc.alloc_tile_pool(name="psE", bufs=3, space="PSUM")

        for t in range(8):
            at = attn[:, t, :]
            st = xz2.tile([P, 8], F32, tag="stats2")
            musum, mu, vsum = st[:, 0:1], st[:, 1:2], st[:, 2:3]
            veps, sdv, rstd = st[:, 4:5], st[:, 5:6], st[:, 6:7]
            nc.vector.reduce_sum(musum, asum[:, t, :], axis=AX.X)
            nc.vector.tensor_scalar_mul(mu, musum, 1.0 / D)
            z = xz2.tile([P, D], BF16, tag="zE")
            zf = xz2.tile([P, D], F32, tag="zEf")
            nc.vector.scalar_tensor_tensor(
                zf[:], at, mu, at,
                op0=ALU.subtract, op1=ALU.mult, accum_out=vsum)
            nc.vector.tensor_scalar(veps, vsum, 1.0 / D, EPS,
                                    op0=ALU.mult, op1=ALU.add)
            nc.scalar.sqrt(sdv, veps)
            nc.vector.reciprocal(rstd, sdv)
            nc.vector.tensor_scalar(z[:], at, mu, rstd,
                                    op0=ALU.subtract, op1=ALU.mult)
            for c in range(8):
                tp = psE.tile([P, P], BF16, tag="tpE")
                nc.tensor.transpose(tp[:], z[:, c * P:(c + 1) * P], identB[:])
                nc.scalar.activation(z2T[:, c, t * P:(t + 1) * P], tp[:],
                                     AF.Identity, bias=b2dm[:, c:c + 1],
                                     scale=g2dm[:, c:c + 1])

        psE.release()
        xz2.release()
        attnp.release()

        # ---------------- Phase F: MLP ---------------------------------------
        h2p = tc.alloc_tile_pool(name="h2acc", bufs=1, side="left")
        h2acc = h2p.tile([P, 8, WINDOW], F32)
        xinTp = tc.alloc_tile_pool(name="xinT", bufs=1, side="left")
        xinT = xinTp.tile([P, 8, WINDOW], F32)
        nc.sync.dma_start(xinT[:], xinT_d.rearrange("(c p) n -> p c n", p=P))
        wf1 = tc.alloc_tile_pool(name="wf1", bufs=1, side="right")
        wf2 = tc.alloc_tile_pool(name="wf2", bufs=1, side="right")
        h1p = tc.alloc_tile_pool(name="h1p", bufs=1, side="left")
        psF1 = tc.alloc_tile_pool(name="psF1", bufs=4, space="PSUM")
        psF2 = tc.alloc_tile_pool(name="psF2", bufs=4, space="PSUM")

        for sc in range(4):
            w1r = wf1.tile([P, 8, 1024], BF16, tag="w1r")
            for kc in range(8):
                nc.sync.dma_start(
                    w1r[:, kc, :],
                    w1_d[kc * P:(kc + 1) * P, sc * 1024:(sc + 1) * 1024])
            h1 = h1p.tile([P, 8, WINDOW], BF16, tag="h1")
            for ft in range(8):
                for qh in range(2):
                    hp = psF1.tile([P, 512], F32, tag="h1ps")
                    for kc in range(8):
                        nc.tensor.matmul(
                            hp[:], w1r[:, kc, ft * P:(ft + 1) * P],
                            z2T[:, kc, qh * 512:(qh + 1) * 512],
                            start=(kc == 0), stop=(kc == 7))
                    nc.scalar.activation(
                        h1[:, ft, qh * 512:(qh + 1) * 512], hp[:], AF.Silu,
                        bias=b1s[:, sc * 8 + ft:sc * 8 + ft + 1], scale=1.0)
            w2r = wf2.tile([P, 8, 1024], BF16, tag="w2r")
            for kc in range(8):
                nc.sync.dma_start(
                    w2r[:, kc, :],
                    w2_d[(sc * 8 + kc) * P:(sc * 8 + kc + 1) * P, :])
            for co in range(8):
                for qh in range(2):
                    hp2 = psF2.tile([P, 512], F32, tag="h2ps")
                    for kc in range(8):
                        nc.tensor.matmul(
                            hp2[:], w2r[:, kc, co * P:(co + 1) * P],
                            h1[:, kc, qh * 512:(qh + 1) * 512],
                            start=(kc == 0), stop=(kc == 7))
                    dstp = h2acc[:, co, qh * 512:(qh + 1) * 512]
                    if sc == 0:
                        nc.vector.tensor_copy(dstp, hp2[:])
                    elif sc < 3:
                        nc.vector.tensor_tensor(dstp, hp2[:], dstp,
                                                op=ALU.add)
                    else:
                        nc.vector.scalar_tensor_tensor(
                            dstp, hp2[:], b2s[:, co:co + 1], dstp,
                            op0=ALU.add, op1=ALU.add)

        psF2.release()
        psF1.release()
        h1p.release()
        wf2.release()
        wf1.release()
        z2Tp.release()

        # tail: residual add in dim-major (host supplies x^T and
        # transposes y back), no PE transposes needed
        tailp = tc.alloc_tile_pool(name="tail", bufs=3, side="left")
        for co in range(8):
            y = tailp.tile([P, WINDOW], F32, tag="y")
            nc.vector.tensor_tensor(y[:], h2acc[:, co, :], xinT[:, co, :],
                                    op=ALU.add)
            nc.sync.dma_start(y_d[co * P:(co + 1) * P, :], y[:])
        tailp.release()
        xinTp.release()
        h2p.release()
        cpool.release()

    nc.compile()
    return nc


def _prep_inputs(inputs):
    x = np.ascontiguousarray(np.asarray(inputs["x"], dtype=np.float32))
    kpm = np.asarray(inputs["key_pad_mask"]).astype(bool)
    bq = np.asarray(inputs["bq"], dtype=np.float32)
    bkv = np.asarray(inputs["bkv"], dtype=np.float32)
    b1 = np.asarray(inputs["b1"], dtype=np.float32)
    b2 = np.asarray(inputs["b2"], dtype=np.float32)
    ln1_g = np.asarray(inputs["ln1_g"], dtype=np.float32)
    ln1_b = np.asarray(inputs["ln1_b"], dtype=np.float32)
    ln2_g = np.asarray(inputs["ln2_g"], dtype=np.float32)
    ln2_b = np.asarray(inputs["ln2_b"], dtype=np.float32)

    def bf(v):
        return np.ascontiguousarray(
            np.asarray(v, dtype=np.float32).astype(ml_dtypes.bfloat16))

    def dm(v):  # [D] -> [P, 8] dim-major chunk layout
        return np.ascontiguousarray(v.reshape(8, P).T)

    shared = {
        "wq": bf(inputs["wq"]),
        "wkv": bf(inputs["wkv"]),
        "w1": bf(inputs["w1"]),
        "w2": bf(inputs["w2"]),
        "bqs": np.ascontiguousarray((bq * ISD).reshape(8, P).T),
        "bkvk": dm(bkv[0:D]),
        "bkvvb": np.ascontiguousarray(
            np.broadcast_to(bkv[D:2 * D], (P, D)).astype(np.float32)),
        "g1dm": dm(ln1_g),
        "b1dm": dm(ln1_b),
        "g2dm": dm(ln2_g),
        "b2dm": dm(ln2_b),
        "b1s": np.ascontiguousarray(b1.reshape(32, P).T),
        "b2s": dm(b2),
    }

    ki = np.arange(P)[:, None]   # key index within block (partition/row)
    qi = np.arange(P)[None, :]   # query index within block (free/col)
    tri = np.where(ki > qi, np.float32(MASKVAL), np.float32(KEEPVAL))
    keep = np.full((P, P), np.float32(KEEPVAL), dtype=np.float32)
    full = np.full((P, P), np.float32(MASKVAL), dtype=np.float32)

    in_maps = []
    for core in range(8):
        b, h = core // 2, core % 2
        perm = [2 * s + h for s in range(8)]
        xq = np.ascontiguousarray(
            x[b, 0:WINDOW * 2].reshape(16, P, D)[perm].reshape(WINDOW, D))
        xw = x[b, S - WINDOW:S]
        pad = kpm[b, S - WINDOW:S]
        m = dict(shared)
        m["xin"] = np.ascontiguousarray(np.concatenate([xq, xw], axis=0))
        m["xinT"] = np.ascontiguousarray(xq.T)
        m["padb"] = np.ascontiguousarray(
            (MASKVAL * pad.astype(np.float32)).reshape(8, P).T)
        m["maskE"] = (tri if h == 0 else keep).astype(ml_dtypes.bfloat16)
        m["maskO"] = (full if h == 0 else tri).astype(ml_dtypes.bfloat16)
        in_maps.append(m)
    return in_maps


def kernel(**inputs):
    from concourse.bass_utils import run_bass_kernel_spmd

    if "nc" not in _CACHE:
        _CACHE["nc"] = _build_program()
    nc = _CACHE["nc"]

    in_maps = _prep_inputs(inputs)
    trace = os.environ.get("KERNEL_TRACE", "0") == "1"
    res = run_bass_kernel_spmd(nc, in_maps, core_ids=list(range(8)),
                               trace=trace)
    if res.exec_time_ns is not None:
        print(f"HW exec time: {res.exec_time_ns} ns")
        _CACHE["exec_time_ns"] = res.exec_time_ns
    out = np.empty((B, S, D), dtype=np.float32)
    for core in range(8):
        b, h = core // 2, core % 2
        yT = res.results[core]["y"].T.reshape(8, P, D)
        dst = out[b, 0:WINDOW * 2].reshape(16, P, D)
        for s in range(8):
            dst[2 * s + h] = yT[s]
    return out


# revision 10
# speedup vs baseline: 1.3580x; 1.2054x over previous
"""Trainium2 Bass kernel for a custom transformer block.

Sharding: 8 cores = 4 batches x 2 interleaved query-chunk sets. Core (b, h)
owns query chunks {2s+h : s in 0..8} (128 tokens each) of batch b; the KV
window (last 1024 tokens) is recomputed on both cores of a batch pair. The
stride-2 interleave balances the causal-triangular attention work across the
pair and lets the score matmuls skip fully-masked key/query blocks: the
score strip for key chunk kc only covers queries from slot kc//2.

All matmul operands are bf16 (fp32 PSUM accumulation); LN stats, softmax
denominators, the MLP accumulator and the residual stay fp32. LN gains and
biases are folded into the weight matrices and projection biases host-side,
so the LN transpose evacuations are batched plain copies (8 transposes into
one PSUM bank, one ACT copy out). Padding masks ride the exp evacuation as
a per-key ACT bias (-80 * pad); the causal diagonal needs a 2D mask only on
the first 128-query block of each key strip, applied post-exp on the bf16
probabilities by the otherwise-idle GpSimd engine (min with exp(-80)/BIG).
Attention is software-pipelined: pair p's score strips (scalar-exp-bound)
interleave with pair p-1's AV matmuls and transposes to keep the PE dense.
Scores are computed transposed ([key, query]); row sums come from an extra
ones-column on V and the normalization is a per-partition vector multiply
after the PE transpose back to token-major.
"""
import sys
import os

if "/opt/trn_rl_repo" not in sys.path:
    sys.path.insert(0, "/opt/trn_rl_repo")

import numpy as np
import ml_dtypes

B, S, D = 4, 2048, 1024
N_HEAD = 16
D_HEAD = 64
WINDOW = 1024
D_FF = 4096
EPS = 1e-5
ISD = float(1.0 / np.sqrt(D))  # 1/32
MASKVAL = -80.0
EXPMASK = float(np.exp(-80.0))  # 1.8e-35: effectively zero, bf16-normal
KEEPVAL = 3e38
P = 128

# first live query slot for key chunk kc (strip start = 128*S_MIN[kc]);
# slot s holds query chunk 2s+h, live when kc <= 2s+h -> s >= ceil((kc-1)/2)
S_MIN = [kc // 2 for kc in range(8)]  # == ceil((kc-1)/2): [0,0,1,1,2,2,3,3]

_CACHE = {}


def _build_program():
    import concourse.bacc as bacc
    import concourse.mybir as mybir
    from concourse.tile import TileContext
    from concourse.masks import make_identity

    F32 = mybir.dt.float32
    BF16 = mybir.dt.bfloat16
    AF = mybir.ActivationFunctionType
    ALU = mybir.AluOpType
    AX = mybir.AxisListType

    nc = bacc.Bacc("TRN2", target_bir_lowering=False, debug=False,
                   num_devices=8)

    xin_d = nc.dram_tensor("xin", [2 * WINDOW, D], F32, kind="ExternalInput")
    wq_d = nc.dram_tensor("wq", [D, D], BF16, kind="ExternalInput")
    wkv_d = nc.dram_tensor("wkv", [D, 2 * D], BF16, kind="ExternalInput")
    w1_d = nc.dram_tensor("w1", [D, D_FF], BF16, kind="ExternalInput")
    w2_d = nc.dram_tensor("w2", [D_FF, D], BF16, kind="ExternalInput")
    bqs_d = nc.dram_tensor("bqs", [P, 8], F32, kind="ExternalInput")
    bkvk_d = nc.dram_tensor("bkvk", [P, 8], F32, kind="ExternalInput")
    bkvvb_d = nc.dram_tensor("bkvvb", [P, D], F32, kind="ExternalInput")
    b1s_d = nc.dram_tensor("b1s", [P, 32], F32, kind="ExternalInput")
    b2s_d = nc.dram_tensor("b2s", [P, 8], F32, kind="ExternalInput")
    padb_d = nc.dram_tensor("padb", [P, 8], F32, kind="ExternalInput")
    maskE_d = nc.dram_tensor("maskE", [P, P], BF16, kind="ExternalInput")
    maskO_d = nc.dram_tensor("maskO", [P, P], BF16, kind="ExternalInput")
    xinT_d = nc.dram_tensor("xinT", [D, WINDOW], F32, kind="ExternalInput")
    y_d = nc.dram_tensor("y", [D, WINDOW], F32, kind="ExternalOutput")

    with TileContext(nc) as tc:
        cpool = tc.alloc_tile_pool(name="const", bufs=1, side="left")
        identB = cpool.tile([P, P], BF16)
        make_identity(nc, identB[:])
        masks = cpool.tile([P, 2 * P], BF16)
        maskE = masks[:, 0:P]
        maskO = masks[:, P:2 * P]
        nc.sync.dma_start(maskE, maskE_d[:])
        nc.sync.dma_start(maskO, maskO_d[:])
        smallc = cpool.tile([P, 80], F32)
        bqs = smallc[:, 0:8]
        bkvk = smallc[:, 8:16]
        b1s = smallc[:, 16:48]
        b2s = smallc[:, 48:56]
        onesc = smallc[:, 56:72]
        padb = smallc[:, 72:80]
        nc.vector.memset(onesc, 1.0)
        nc.sync.dma_start(bqs, bqs_d[:])
        nc.sync.dma_start(bkvk, bkvk_d[:])
        nc.sync.dma_start(b1s, b1s_d[:])
        nc.sync.dma_start(b2s, b2s_d[:])
        nc.sync.dma_start(padb, padb_d[:])

        # ---------------- Phase B/C: LN1 + QKV projections ------------------
        # z = (x - mu) * rstd token-major (LN gain/bias folded into weights);
        # 8 PE transposes batch into one PSUM bank, one ACT copy evacuates.
        # Window tiles (8-15) first so the V/K projections overlap the LN of
        # the query half, keeping the PE dense from the start.
        zTp = tc.alloc_tile_pool(name="zT", bufs=1, side="left")
        zqT = zTp.tile([P, 8, WINDOW], BF16)
        zwT = zTp.tile([P, 8, WINDOW], BF16)
        xz = tc.alloc_tile_pool(name="xz", bufs=3, side="left")
        psB = tc.alloc_tile_pool(name="psB", bufs=3, space="PSUM")

        def ln1_tile(t):
            xt = xz.tile([P, D], F32, tag="x")
            nc.sync.dma_start(xt[:], xin_d[t * P:(t + 1) * P, :])
            st = xz.tile([P, 8], F32, tag="stats")
            musum, mu, vsum = st[:, 0:1], st[:, 1:2], st[:, 2:3]
            veps, sdv, rstd = st[:, 4:5], st[:, 5:6], st[:, 6:7]
            nc.vector.reduce_sum(musum, xt[:], axis=AX.X)
            nc.vector.tensor_scalar_mul(mu, musum, 1.0 / D)
            z = xz.tile([P, D], BF16, tag="z")
            zf = xz.tile([P, D], F32, tag="zf")
            # scratch into zf; vsum = sum((x - mu) * x) = D * var
            nc.vector.scalar_tensor_tensor(
                zf[:], xt[:], mu, xt[:],
                op0=ALU.subtract, op1=ALU.mult, accum_out=vsum)
            nc.vector.tensor_scalar(veps, vsum, 1.0 / D, EPS,
                                    op0=ALU.mult, op1=ALU.add)
            nc.scalar.sqrt(sdv, veps)
            nc.vector.reciprocal(rstd, sdv)
            nc.vector.tensor_scalar(z[:], xt[:], mu, rstd,
                                    op0=ALU.subtract, op1=ALU.mult)
            batch = psB.tile([P, D], BF16, tag="tpB")
            for c in range(8):
                nc.tensor.transpose(batch[:, c * P:(c + 1) * P],
                                    z[:, c * P:(c + 1) * P], identB[:])
            dst = zqT if t < 8 else zwT
            col = (t % 8) * P
            nc.scalar.copy(dst[:, :, col:col + P],
                           batch[:].rearrange("p (c n) -> p c n", n=P))

        for t in range(8, 16):
            ln1_tile(t)

        qkvp = tc.alloc_tile_pool(name="qkv", bufs=1, side="right")
        qT = qkvp.tile([P, 8, WINDOW], BF16)      # q/sqrt(D), dim-major
        kT = qkvp.tile([P, 8, WINDOW], BF16)      # k, dim-major
        V = qkvp.tile([P, 8, N_HEAD * 65], BF16)  # token-major + ones col

        wst = tc.alloc_tile_pool(name="wst", bufs=1, side="left")
        psC = tc.alloc_tile_pool(name="psC", bufs=4, space="PSUM")

        # V: activations stationary -> token-major, bias added via bcast tile
        bkvvb = wst.tile([P, D], F32, tag="bkvvb")
        nc.sync.dma_start(bkvvb[:], bkvvb_d[:])

        def v_proj(vh):
            wvr = wst.tile([P, 8, 512], BF16, tag="wkres")
            for kc in range(8):
                nc.sync.dma_start(
                    wvr[:, kc, :],
                    wkv_d[kc * P:(kc + 1) * P,
                          D + vh * 512:D + (vh + 1) * 512])
            for tt in range(8):
                pp = psC.tile([P, 512], F32, tag="proj")
                for kc in range(8):
                    nc.tensor.matmul(
                        pp[:], zwT[:, kc, tt * P:(tt + 1) * P],
                        wvr[:, kc, :],
                        start=(kc == 0), stop=(kc == 7))
                vdst = V[:, tt, :].rearrange("p (h n) -> p h n", n=65)[
                    :, vh * 8:(vh + 1) * 8, 0:64]
                nc.vector.scalar_tensor_tensor(
                    vdst, pp[:].rearrange("p (h n) -> p h n", n=64), 0.0,
                    bkvvb[:, vh * 512:(vh + 1) * 512].rearrange(
                        "p (h n) -> p h n", n=64),
                    op0=ALU.add, op1=ALU.add)

        def kq_proj(w_d, wh, dst, bias, scale):
            wkr = wst.tile([P, 8, 512], BF16, tag="wkres")
            for kc in range(8):
                nc.sync.dma_start(
                    wkr[:, kc, :],
                    w_d[kc * P:(kc + 1) * P, wh * 512:(wh + 1) * 512])
            src = zwT if dst is kT else zqT
            for co in range(wh * 4, wh * 4 + 4):
                for qh in range(2):
                    pp = psC.tile([P, 512], F32, tag="proj")
                    for kc in range(8):
                        nc.tensor.matmul(
                            pp[:], wkr[:, kc, (co % 4) * P:(co % 4 + 1) * P],
                            src[:, kc, qh * 512:(qh + 1) * 512],
                            start=(kc == 0), stop=(kc == 7))
                    nc.scalar.activation(
                        dst[:, co, qh * 512:(qh + 1) * 512], pp[:],
                        AF.Identity, bias=bias[:, co:co + 1], scale=scale)

        # interleave LN of the query half between V/K/Q projection blocks
        v_proj(0)
        ln1_tile(0); ln1_tile(1)
        v_proj(1)
        for tt in range(8):
            nc.scalar.copy(
                V[:, tt, :].rearrange("p (h n) -> p h n", n=65)[:, :, 64:65],
                onesc.rearrange("p (h n) -> p h n", n=1))
        ln1_tile(2); ln1_tile(3)
        kq_proj(wkv_d, 0, kT, bkvk, 1.0)
        ln1_tile(4); ln1_tile(5)
        kq_proj(wkv_d, 1, kT, bkvk, 1.0)
        ln1_tile(6); ln1_tile(7)
        kq_proj(wq_d, 0, qT, bqs, ISD)
        kq_proj(wq_d, 1, qT, bqs, ISD)

        psC.release()
        psB.release()
        wst.release()
        xz.release()
        zTp.release()

        # ---------------- Phase D: attention (software-pipelined) -----------
        attnp = tc.alloc_tile_pool(name="attn", bufs=1, side="left")
        attn = attnp.tile([P, 8, D], F32)          # normalized attn out
        rinva = attnp.tile([P, 8, N_HEAD], F32)    # per-head 1/rowsum
        # ping-pong exp(score) tiles: [pair-head, kc, query]; the columns
        # before each strip start are never written -> zero them once
        ptsA = attnp.tile([P, 2, 8, WINDOW], BF16)
        ptsB = attnp.tile([P, 2, 8, WINDOW], BF16)
        for pts in (ptsA, ptsB):
            for h2 in range(2):
                for kc in range(2, 8):
                    z0 = S_MIN[kc] * P
                    nc.vector.memset(pts[:, h2, kc, 0:z0], 0.0)

        sbD = tc.alloc_tile_pool(name="sbD", bufs=2, side="left")
        psDs = tc.alloc_tile_pool(name="psDs", bufs=2, space="PSUM")
        psDa = tc.alloc_tile_pool(name="psDa", bufs=2, space="PSUM")
        psDt = tc.alloc_tile_pool(name="psDt", bufs=2, space="PSUM")

        oa_tiles = {}

        def score_block(pts, pair, kc):
            z0 = S_MIN[kc] * P
            L = WINDOW - z0
            strips = {}
            for h in pair:
                strips[h] = psDs.tile([P, WINDOW], F32, tag="s", name="s")
            for c0, c1 in ((0, 512), (512, L)):
                for h in pair:
                    po, ch = (h % 2) * 64, h // 2
                    nc.tensor.matmul(
                        strips[h][:, c0:c1],
                        kT[po:po + 64, ch, kc * P:(kc + 1) * P],
                        qT[po:po + 64, ch, z0 + c0:z0 + c1],
                        start=True, stop=True)
            mk = maskE if kc % 2 == 0 else maskO
            for h in pair:
                nc.scalar.activation(
                    pts[:, h % 2, kc, z0:WINDOW], strips[h][:, 0:L],
                    AF.Exp, bias=padb[:, kc:kc + 1])
                # causal diagonal / dead block: min on the bf16 probs
                # (exp is monotone); all-SBUF 16-bit op -> 2x DVE rate
                nc.vector.tensor_tensor(
                    pts[:, h % 2, kc, z0:z0 + P],
                    pts[:, h % 2, kc, z0:z0 + P], mk, op=ALU.min)

        def av_chain(pts, h, qh):
            if qh == 0:
                oa_tiles[h] = sbD.tile([65, WINDOW], BF16, tag="oa",
                                       name="oa")
            avp = psDa.tile([65, 512], F32, tag="av")
            for kc in range(8):
                nc.tensor.matmul(
                    avp[:], V[:, kc, h * 65:(h + 1) * 65],
                    pts[:, h % 2, kc, qh * 512:(qh + 1) * 512],
                    start=(kc == 0), stop=(kc == 7))
            nc.vector.tensor_copy(oa_tiles[h][:, qh * 512:(qh + 1) * 512],
                                  avp[:])

        def trans_block(h):
            oa = oa_tiles[h]
            # 66-wide groups keep each transpose output 4-byte aligned
            tpb = psDt.tile([P, 8, 66], BF16, tag="tp65")
            for t in range(8):
                nc.tensor.transpose(tpb[:, t, 0:65],
                                    oa[:, t * P:(t + 1) * P],
                                    identB[0:65, 0:65])
            for t in range(8):
                rinv = rinva[:, t, h:h + 1]
                nc.vector.reciprocal(rinv, tpb[:, t, 64:65])
                nc.vector.tensor_scalar_mul(
                    attn[:, t, h * 64:(h + 1) * 64], tpb[:, t, 0:64], rinv)

        for p in range(9):
            prev = []
            if p > 0:
                pv = ptsA if (p - 1) % 2 == 0 else ptsB
                for h in (2 * (p - 1), 2 * (p - 1) + 1):
                    prev.append(lambda h=h, pv=pv: av_chain(pv, h, 0))
                    prev.append(lambda h=h, pv=pv: av_chain(pv, h, 1))
                    prev.append(lambda h=h: trans_block(h))
            if p < 8:
                pts = ptsA if p % 2 == 0 else ptsB
                pair = (2 * p, 2 * p + 1)
                for kc in range(8):
                    score_block(pts, pair, kc)
                    if kc < len(prev):
                        prev[kc]()
            else:
                for task in prev:
                    task()

        psDt.release()
        psDa.release()
        psDs.release()
        sbD.release()
        qkvp.release()

        # ---------------- Phase E: LN2 + transpose ---------------------------
        z2Tp = tc.alloc_tile_pool(name="z2T", bufs=1, side="right")
        z2T = z2Tp.tile([P, 8, WINDOW], BF16)
        xz2 = tc.alloc_tile_pool(name="xz2", bufs=3, side="left")
        psE = tc.alloc_tile_pool(name="psE", bufs=3, space="PSUM")

        for t in range(8):
            at = attn[:, t, :]
            st = xz2.tile([P, 8], F32, tag="stats2")
            musum, mu, vsum = st[:, 0:1], st[:, 1:2], st[:, 2:3]
            veps, sdv, rstd = st[:, 4:5], st[:, 5:6], st[:, 6:7]
            nc.vector.reduce_sum(musum, at, axis=AX.X)
            nc.vector.tensor_scalar_mul(mu, musum, 1.0 / D)
            z = xz2.tile([P, D], BF16, tag="zE")
            zf = xz2.tile([P, D], F32, tag="zEf")
            nc.vector.scalar_tensor_tensor(
                zf[:], at, mu, at,
                op0=ALU.subtract, op1=ALU.mult, accum_out=vsum)
            nc.vector.tensor_scalar(veps, vsum, 1.0 / D, EPS,
                                    op0=ALU.mult, op1=ALU.add)
            nc.scalar.sqrt(sdv, veps)
            nc.vector.reciprocal(rstd, sdv)
            nc.vector.tensor_scalar(z[:], at, mu, rstd,
                                    op0=ALU.subtract, op1=ALU.mult)
            batch = psE.tile([P, D], BF16, tag="tpE")
            for c in range(8):
                nc.tensor.transpose(batch[:, c * P:(c + 1) * P],
                                    z[:, c * P:(c + 1) * P], identB[:])
            nc.scalar.copy(z2T[:, :, t * P:(t + 1) * P],
                           batch[:].rearrange("p (c n) -> p c n", n=P))

        psE.release()
        xz2.release()
        attnp.release()

        # ---------------- Phase F: MLP + residual ----------------------------
        h2p = tc.alloc_tile_pool(name="h2acc", bufs=1, side="left")
        h2acc = h2p.tile([P, 8, WINDOW], F32)
        xinTp = tc.alloc_tile_pool(name="xinT", bufs=1, side="left")
        xinT = xinTp.tile([P, 8, WINDOW], F32)
        nc.sync.dma_start(xinT[:], xinT_d.rearrange("(c p) n -> p c n", p=P))
        wf1 = tc.alloc_tile_pool(name="wf1", bufs=1, side="right")
        wf2 = tc.alloc_tile_pool(name="wf2", bufs=1, side="right")
        h1p = tc.alloc_tile_pool(name="h1p", bufs=1, side="left")
        tailp = tc.alloc_tile_pool(name="tail", bufs=3, side="left")
        psF1 = tc.alloc_tile_pool(name="psF1", bufs=4, space="PSUM")
        psF2 = tc.alloc_tile_pool(name="psF2", bufs=4, space="PSUM")

        for sc in range(4):
            w1r = wf1.tile([P, 8, 1024], BF16, tag="w1r")
            for kc in range(8):
                nc.sync.dma_start(
                    w1r[:, kc, :],
                    w1_d[kc * P:(kc + 1) * P, sc * 1024:(sc + 1) * 1024])
            h1 = h1p.tile([P, 8, WINDOW], BF16, tag="h1")
            for ft in range(8):
                for qh in range(2):
                    hp = psF1.tile([P, 512], F32, tag="h1ps")
                    for kc in range(8):
                        nc.tensor.matmul(
                            hp[:], w1r[:, kc, ft * P:(ft + 1) * P],
                            z2T[:, kc, qh * 512:(qh + 1) * 512],
                            start=(kc == 0), stop=(kc == 7))
                    nc.scalar.activation(
                        h1[:, ft, qh * 512:(qh + 1) * 512], hp[:], AF.Silu,
                        bias=b1s[:, sc * 8 + ft:sc * 8 + ft + 1], scale=1.0)
            w2r = wf2.tile([P, 8, 1024], BF16, tag="w2r")
            for kc in range(8):
                nc.sync.dma_start(
                    w2r[:, kc, :],
                    w2_d[(sc * 8 + kc) * P:(sc * 8 + kc + 1) * P, :])
            for co in range(8):
                for qh in range(2):
                    hp2 = psF2.tile([P, 512], F32, tag="h2ps")
                    for kc in range(8):
                        nc.tensor.matmul(
                            hp2[:], w2r[:, kc, co * P:(co + 1) * P],
                            h1[:, kc, qh * 512:(qh + 1) * 512],
                            start=(kc == 0), stop=(kc == 7))
                    dstp = h2acc[:, co, qh * 512:(qh + 1) * 512]
                    if sc == 0:
                        nc.vector.tensor_copy(dstp, hp2[:])
                    elif sc < 3:
                        nc.vector.tensor_tensor(dstp, hp2[:], dstp,
                                                op=ALU.add)
                    else:
                        nc.vector.scalar_tensor_tensor(
                            dstp, hp2[:], b2s[:, co:co + 1], dstp,
                            op0=ALU.add, op1=ALU.add)
                if sc == 3:
                    # residual add + store as soon as this dim chunk is done
                    y = tailp.tile([P, WINDOW], F32, tag="y")
                    nc.vector.tensor_tensor(y[:], h2acc[:, co, :],
                                            xinT[:, co, :], op=ALU.add)
                    nc.sync.dma_start(y_d[co * P:(co + 1) * P, :], y[:])

        psF2.release()
        psF1.release()
        tailp.release()
        h1p.release()
        wf2.release()
        wf1.release()
        z2Tp.release()
        xinTp.release()
        h2p.release()
        cpool.release()

    nc.compile()
    return nc


def _prep_inputs(inputs):
    x = np.ascontiguousarray(np.asarray(inputs["x"], dtype=np.float32))
    kpm = np.asarray(inputs["key_pad_mask"]).astype(bool)
    wq = np.asarray(inputs["wq"], dtype=np.float32)
    wkv = np.asarray(inputs["wkv"], dtype=np.float32)
    w1 = np.asarray(inputs["w1"], dtype=np.float32)
    w2 = np.asarray(inputs["w2"], dtype=np.float32)
    bq = np.asarray(inputs["bq"], dtype=np.float32)
    bkv = np.asarray(inputs["bkv"], dtype=np.float32)
    b1 = np.asarray(inputs["b1"], dtype=np.float32)
    b2 = np.asarray(inputs["b2"], dtype=np.float32)
    ln1_g = np.asarray(inputs["ln1_g"], dtype=np.float32)
    ln1_b = np.asarray(inputs["ln1_b"], dtype=np.float32)
    ln2_g = np.asarray(inputs["ln2_g"], dtype=np.float32)
    ln2_b = np.asarray(inputs["ln2_b"], dtype=np.float32)

    # fold the LN affine transforms into the weights/biases (host-side):
    # (z*g + b) @ W + c == z @ (diag(g) W) + (b @ W + c)
    wq_f = ln1_g[:, None] * wq
    bq_f = ln1_b @ wq + bq
    wkv_f = ln1_g[:, None] * wkv
    bkv_f = ln1_b @ wkv + bkv
    w1_f = ln2_g[:, None] * w1
    b1_f = ln2_b @ w1 + b1

    def bf(v):
        return np.ascontiguousarray(v.astype(ml_dtypes.bfloat16))

    def dm(v):  # [D] -> [P, 8] dim-major chunk layout
        return np.ascontiguousarray(v.reshape(8, P).T)

    shared = {
        "wq": bf(wq_f),
        "wkv": bf(wkv_f),
        "w1": bf(w1_f),
        "w2": bf(w2),
        "bqs": np.ascontiguousarray((bq_f * ISD).reshape(8, P).T),
        "bkvk": dm(bkv_f[0:D]),
        "bkvvb": np.ascontiguousarray(
            np.broadcast_to(bkv_f[D:2 * D], (P, D)).astype(np.float32)),
        "b1s": np.ascontiguousarray(b1_f.reshape(32, P).T),
        "b2s": dm(b2),
    }

    ki = np.arange(P)[:, None]   # key index within block (partition/row)
    qi = np.arange(P)[None, :]   # query index within block (free/col)
    tri = np.where(ki > qi, np.float32(EXPMASK), np.float32(KEEPVAL))
    keep = np.full((P, P), np.float32(KEEPVAL), dtype=np.float32)
    full = np.full((P, P), np.float32(EXPMASK), dtype=np.float32)

    in_maps = []
    for core in range(8):
        b, h = core // 2, core % 2
        perm = [2 * s + h for s in range(8)]
        xq = np.ascontiguousarray(
            x[b, 0:WINDOW * 2].reshape(16, P, D)[perm].reshape(WINDOW, D))
        xw = x[b, S - WINDOW:S]
        pad = kpm[b, S - WINDOW:S]
        m = dict(shared)
        m["xin"] = np.ascontiguousarray(np.concatenate([xq, xw], axis=0))
        m["xinT"] = np.ascontiguousarray(xq.T)
        m["padb"] = np.ascontiguousarray(
            (MASKVAL * pad.astype(np.float32)).reshape(8, P).T)
        m["maskE"] = (tri if h == 0 else keep).astype(ml_dtypes.bfloat16)
        m["maskO"] = (full if h == 0 else tri).astype(ml_dtypes.bfloat16)
        in_maps.append(m)
    return in_maps


def kernel(**inputs):
    from concourse.bass_utils import run_bass_kernel_spmd

    if "nc" not in _CACHE:
        _CACHE["nc"] = _build_program()
    nc = _CACHE["nc"]

    in_maps = _prep_inputs(inputs)
    trace = os.environ.get("KERNEL_TRACE", "0") == "1"
    res = run_bass_kernel_spmd(nc, in_maps, core_ids=list(range(8)),
                               trace=trace)
    if res.exec_time_ns is not None:
        print(f"HW exec time: {res.exec_time_ns} ns")
        _CACHE["exec_time_ns"] = res.exec_time_ns
    out = np.empty((B, S, D), dtype=np.float32)
    for core in range(8):
        b, h = core // 2, core % 2
        yT = res.results[core]["y"].T.reshape(8, P, D)
        dst = out[b, 0:WINDOW * 2].reshape(16, P, D)
        for s in range(8):
            dst[2 * s + h] = yT[s]
    return out


# revision 14
# speedup vs baseline: 1.3681x; 1.0074x over previous
"""Trainium2 Bass kernel for a custom transformer block.

Sharding: 8 cores = 4 batches x 2 interleaved query-chunk sets. Core (b, h)
owns query chunks {2s+h : s in 0..8} (128 tokens each) of batch b; the KV
window (last 1024 tokens) is recomputed on both cores of a batch pair. The
stride-2 interleave balances the causal-triangular attention work across the
pair and lets the score matmuls skip fully-masked key/query blocks: the
score strip for key chunk kc only covers queries from slot kc//2.

All matmul operands are bf16 (fp32 PSUM accumulation); LN stats, softmax
denominators, the MLP accumulator and the residual stay fp32. LN gains and
biases are folded into the weight matrices and projection biases host-side,
so the LN transpose evacuations are batched plain copies (8 transposes into
one PSUM bank, one ACT copy out). Padding masks ride the exp evacuation as
a per-key ACT bias (-80 * pad); the causal diagonal needs a 2D mask only on
the first 128-query block of each key strip, applied post-exp on the bf16
probabilities by the otherwise-idle GpSimd engine (min with exp(-80)/BIG).
Attention is software-pipelined: pair p's score strips (scalar-exp-bound)
interleave with pair p-1's AV matmuls and transposes to keep the PE dense.
Scores are computed transposed ([key, query]); row sums come from an extra
ones-column on V and the normalization is a per-partition vector multiply
after the PE transpose back to token-major.
"""
import sys
import os

if "/opt/trn_rl_repo" not in sys.path:
    sys.path.insert(0, "/opt/trn_rl_repo")

import numpy as np
import ml_dtypes

B, S, D = 4, 2048, 1024
N_HEAD = 16
D_HEAD = 64
WINDOW = 1024
D_FF = 4096
EPS = 1e-5
ISD = float(1.0 / np.sqrt(D))  # 1/32
MASKVAL = -80.0
EXPMASK = float(np.exp(-80.0))  # 1.8e-35: effectively zero, bf16-normal
KEEPVAL = 3e38
P = 128

# first live query slot for key chunk kc (strip start = 128*S_MIN[kc]);
# slot s holds query chunk 2s+h, live when kc <= 2s+h -> s >= ceil((kc-1)/2)
S_MIN = [kc // 2 for kc in range(8)]  # == ceil((kc-1)/2): [0,0,1,1,2,2,3,3]

_CACHE = {}


def _build_program():
    import concourse.bacc as bacc
    import concourse.mybir as mybir
    from concourse.tile import TileContext
    from concourse.masks import make_identity

    F32 = mybir.dt.float32
    BF16 = mybir.dt.bfloat16
    AF = mybir.ActivationFunctionType
    ALU = mybir.AluOpType
    AX = mybir.AxisListType

    nc = bacc.Bacc("TRN2", target_bir_lowering=False, debug=False,
                   num_devices=8)

    xin_d = nc.dram_tensor("xin", [2 * WINDOW, D], F32, kind="ExternalInput")
    wq_d = nc.dram_tensor("wq", [D, D], BF16, kind="ExternalInput")
    wkv_d = nc.dram_tensor("wkv", [D, 2 * D], BF16, kind="ExternalInput")
    w1_d = nc.dram_tensor("w1", [D, D_FF], BF16, kind="ExternalInput")
    w2_d = nc.dram_tensor("w2", [D_FF, D], BF16, kind="ExternalInput")
    bqs_d = nc.dram_tensor("bqs", [P, 8], F32, kind="ExternalInput")
    bkvk_d = nc.dram_tensor("bkvk", [P, 8], F32, kind="ExternalInput")
    bkvvb_d = nc.dram_tensor("bkvvb", [P, D], F32, kind="ExternalInput")
    b1s_d = nc.dram_tensor("b1s", [P, 32], F32, kind="ExternalInput")
    b2s_d = nc.dram_tensor("b2s", [P, 8], F32, kind="ExternalInput")
    padb_d = nc.dram_tensor("padb", [P, 8], F32, kind="ExternalInput")
    maskE_d = nc.dram_tensor("maskE", [P, P], BF16, kind="ExternalInput")
    maskO_d = nc.dram_tensor("maskO", [P, P], BF16, kind="ExternalInput")
    xinT_d = nc.dram_tensor("xinT", [D, WINDOW], F32, kind="ExternalInput")
    y_d = nc.dram_tensor("y", [D, WINDOW], F32, kind="ExternalOutput")

    with TileContext(nc) as tc:
        cpool = tc.alloc_tile_pool(name="const", bufs=1, side="left")
        identB = cpool.tile([P, P], BF16)
        make_identity(nc, identB[:])
        masks = cpool.tile([P, 2 * P], BF16)
        maskE = masks[:, 0:P]
        maskO = masks[:, P:2 * P]
        nc.sync.dma_start(maskE, maskE_d[:])
        nc.sync.dma_start(maskO, maskO_d[:])
        smallc = cpool.tile([P, 80], F32)
        bqs = smallc[:, 0:8]
        bkvk = smallc[:, 8:16]
        b1s = smallc[:, 16:48]
        b2s = smallc[:, 48:56]
        onesc = smallc[:, 56:72]
        padb = smallc[:, 72:80]
        nc.vector.memset(onesc, 1.0)
        nc.sync.dma_start(bqs, bqs_d[:])
        nc.sync.dma_start(bkvk, bkvk_d[:])
        nc.sync.dma_start(b1s, b1s_d[:])
        nc.sync.dma_start(b2s, b2s_d[:])
        nc.sync.dma_start(padb, padb_d[:])

        # ---------------- Phase B/C: LN1 + QKV projections ------------------
        # z = (x - mu) * rstd token-major (LN gain/bias folded into weights);
        # 8 PE transposes batch into one PSUM bank, one ACT copy evacuates.
        # Window tiles (8-15) first so the V/K projections overlap the LN of
        # the query half, keeping the PE dense from the start.
        zTp = tc.alloc_tile_pool(name="zT", bufs=1, side="left")
        zqT = zTp.tile([P, 8, WINDOW], BF16)
        zwT = zTp.tile([P, 8, WINDOW], BF16)
        xz = tc.alloc_tile_pool(name="xz", bufs=3, side="left")
        psB = tc.alloc_tile_pool(name="psB", bufs=3, space="PSUM")

        def ln1_tile(t):
            xt = xz.tile([P, D], F32, tag="x")
            nc.sync.dma_start(xt[:], xin_d[t * P:(t + 1) * P, :])
            st = xz.tile([P, 8], F32, tag="stats")
            musum, mu, vsum = st[:, 0:1], st[:, 1:2], st[:, 2:3]
            veps, sdv, rstd = st[:, 4:5], st[:, 5:6], st[:, 6:7]
            nc.vector.reduce_sum(musum, xt[:], axis=AX.X)
            nc.vector.tensor_scalar_mul(mu, musum, 1.0 / D)
            z = xz.tile([P, D], BF16, tag="z")
            zf = xz.tile([P, D], F32, tag="zf")
            # scratch into zf; vsum = sum((x - mu) * x) = D * var
            nc.vector.scalar_tensor_tensor(
                zf[:], xt[:], mu, xt[:],
                op0=ALU.subtract, op1=ALU.mult, accum_out=vsum)
            nc.vector.tensor_scalar(veps, vsum, 1.0 / D, EPS,
                                    op0=ALU.mult, op1=ALU.add)
            nc.scalar.sqrt(sdv, veps)
            nc.vector.reciprocal(rstd, sdv)
            nc.vector.tensor_scalar(z[:], xt[:], mu, rstd,
                                    op0=ALU.subtract, op1=ALU.mult)
            batch = psB.tile([P, D], BF16, tag="tpB")
            for c in range(8):
                nc.tensor.transpose(batch[:, c * P:(c + 1) * P],
                                    z[:, c * P:(c + 1) * P], identB[:])
            dst = zqT if t < 8 else zwT
            col = (t % 8) * P
            nc.scalar.copy(dst[:, :, col:col + P],
                           batch[:].rearrange("p (c n) -> p c n", n=P))

        for t in range(8, 16):
            ln1_tile(t)

        qkvp = tc.alloc_tile_pool(name="qkv", bufs=1, side="right")
        qT = qkvp.tile([P, 8, WINDOW], BF16)      # q/sqrt(D), dim-major
        kT = qkvp.tile([P, 8, WINDOW], BF16)      # k, dim-major
        V = qkvp.tile([P, 8, N_HEAD * 65], BF16)  # token-major + ones col

        wst = tc.alloc_tile_pool(name="wst", bufs=1, side="left")
        psC = tc.alloc_tile_pool(name="psC", bufs=4, space="PSUM")

        # V: activations stationary -> token-major, bias added via bcast tile
        bkvvb = wst.tile([P, D], F32, tag="bkvvb")
        nc.sync.dma_start(bkvvb[:], bkvvb_d[:])

        def wload(w_d, c0):  # [D, 512] weight block, dim-chunked
            wr = wst.tile([P, 8, 512], BF16, tag="wkres", bufs=4, name="wr")
            for kc in range(8):
                nc.sync.dma_start(
                    wr[:, kc, :], w_d[kc * P:(kc + 1) * P, c0:c0 + 512])
            return wr

        def v_chain(tt, vh, wvr):
            pp = psC.tile([P, 512], F32, tag="proj")
            for kc in range(8):
                nc.tensor.matmul(
                    pp[:], zwT[:, kc, tt * P:(tt + 1) * P],
                    wvr[:, kc, :],
                    start=(kc == 0), stop=(kc == 7))
            vdst = V[:, tt, :].rearrange("p (h n) -> p h n", n=65)[
                :, vh * 8:(vh + 1) * 8, 0:64]
            nc.vector.scalar_tensor_tensor(
                vdst, pp[:].rearrange("p (h n) -> p h n", n=64), 0.0,
                bkvvb[:, vh * 512:(vh + 1) * 512].rearrange(
                    "p (h n) -> p h n", n=64),
                op0=ALU.add, op1=ALU.add)

        def kq_half(wkr, wh, qh, dst, src, bias, scale):
            for co in range(wh * 4, wh * 4 + 4):
                pp = psC.tile([P, 512], F32, tag="proj")
                for kc in range(8):
                    nc.tensor.matmul(
                        pp[:], wkr[:, kc, (co % 4) * P:(co % 4 + 1) * P],
                        src[:, kc, qh * 512:(qh + 1) * 512],
                        start=(kc == 0), stop=(kc == 7))
                nc.scalar.activation(
                    dst[:, co, qh * 512:(qh + 1) * 512], pp[:],
                    AF.Identity, bias=bias[:, co:co + 1], scale=scale)

        # window LN first; V token-chunk chains interleave with it, then
        # K (window-only) and Q interleave with the query-half LN
        wvr0 = wload(wkv_d, D)
        wvr1 = wload(wkv_d, D + 512)
        ln1_tile(8)
        ln1_tile(9)
        for tt in range(8):
            if tt + 10 < 16:
                ln1_tile(tt + 10)
            v_chain(tt, 0, wvr0)
            v_chain(tt, 1, wvr1)
        for tt in range(8):
            nc.scalar.copy(
                V[:, tt, :].rearrange("p (h n) -> p h n", n=65)[:, :, 64:65],
                onesc.rearrange("p (h n) -> p h n", n=1))
        wkr0 = wload(wkv_d, 0)
        wkr1 = wload(wkv_d, 512)
        ln1_tile(0)
        kq_half(wkr0, 0, 0, kT, zwT, bkvk, 1.0)
        ln1_tile(1)
        kq_half(wkr0, 0, 1, kT, zwT, bkvk, 1.0)
        ln1_tile(2)
        kq_half(wkr1, 1, 0, kT, zwT, bkvk, 1.0)
        ln1_tile(3)
        kq_half(wkr1, 1, 1, kT, zwT, bkvk, 1.0)
        wqr0 = wload(wq_d, 0)
        ln1_tile(4)
        kq_half(wqr0, 0, 0, qT, zqT, bqs, ISD)
        wqr1 = wload(wq_d, 512)
        ln1_tile(5)
        kq_half(wqr1, 1, 0, qT, zqT, bqs, ISD)
        ln1_tile(6)
        ln1_tile(7)
        kq_half(wqr0, 0, 1, qT, zqT, bqs, ISD)
        kq_half(wqr1, 1, 1, qT, zqT, bqs, ISD)

        psC.release()
        psB.release()
        wst.release()
        xz.release()
        zTp.release()

        # ---------------- Phase D: attention (software-pipelined) -----------
        attnp = tc.alloc_tile_pool(name="attn", bufs=1, side="left")
        attn = attnp.tile([P, 8, D], F32)          # normalized attn out
        rinva = attnp.tile([P, 8, N_HEAD], F32)    # per-head 1/rowsum
        # ping-pong exp(score) tiles: [pair-head, kc, query]; the columns
        # before each strip start are never written -> zero them once
        ptsA = attnp.tile([P, 2, 8, WINDOW], BF16)
        ptsB = attnp.tile([P, 2, 8, WINDOW], BF16)
        for pts in (ptsA, ptsB):
            for h2 in range(2):
                for kc in range(2, 8):
                    z0 = S_MIN[kc] * P
                    nc.vector.memset(pts[:, h2, kc, 0:z0], 0.0)

        sbD = tc.alloc_tile_pool(name="sbD", bufs=2, side="left")
        psDs = tc.alloc_tile_pool(name="psDs", bufs=2, space="PSUM")
        psDa = tc.alloc_tile_pool(name="psDa", bufs=2, space="PSUM")
        psDt = tc.alloc_tile_pool(name="psDt", bufs=2, space="PSUM")

        oa_tiles = {}

        def score_block(pts, pair, kc):
            z0 = S_MIN[kc] * P
            L = WINDOW - z0
            strips = {}
            for h in pair:
                strips[h] = psDs.tile([P, WINDOW], F32, tag="s", name="s")
            for c0, c1 in ((0, 512), (512, L)):
                for h in pair:
                    po, ch = (h % 2) * 64, h // 2
                    nc.tensor.matmul(
                        strips[h][:, c0:c1],
                        kT[po:po + 64, ch, kc * P:(kc + 1) * P],
                        qT[po:po + 64, ch, z0 + c0:z0 + c1],
                        start=True, stop=True)
            mk = maskE if kc % 2 == 0 else maskO
            for h in pair:
                nc.scalar.activation(
                    pts[:, h % 2, kc, z0:WINDOW], strips[h][:, 0:L],
                    AF.Exp, bias=padb[:, kc:kc + 1])
                # causal diagonal / dead block: min on the bf16 probs
                # (exp is monotone); all-SBUF 16-bit op -> 2x DVE rate
                nc.vector.tensor_tensor(
                    pts[:, h % 2, kc, z0:z0 + P],
                    pts[:, h % 2, kc, z0:z0 + P], mk, op=ALU.min)

        def av_chain(pts, h, qh):
            if qh == 0:
                oa_tiles[h] = sbD.tile([65, WINDOW], BF16, tag="oa",
                                       name="oa")
            avp = psDa.tile([65, 512], F32, tag="av")
            for kc in range(8):
                nc.tensor.matmul(
                    avp[:], V[:, kc, h * 65:(h + 1) * 65],
                    pts[:, h % 2, kc, qh * 512:(qh + 1) * 512],
                    start=(kc == 0), stop=(kc == 7))
            nc.vector.tensor_copy(oa_tiles[h][:, qh * 512:(qh + 1) * 512],
                                  avp[:])

        def trans_block(h):
            oa = oa_tiles[h]
            # 66-wide groups keep each transpose output 4-byte aligned
            tpb = psDt.tile([P, 8, 66], BF16, tag="tp65")
            for t in range(8):
                nc.tensor.transpose(tpb[:, t, 0:65],
                                    oa[:, t * P:(t + 1) * P],
                                    identB[0:65, 0:65])
            for t in range(8):
                rinv = rinva[:, t, h:h + 1]
                nc.vector.reciprocal(rinv, tpb[:, t, 64:65])
                nc.vector.tensor_scalar_mul(
                    attn[:, t, h * 64:(h + 1) * 64], tpb[:, t, 0:64], rinv)

        for p in range(9):
            prev = []
            if p > 0:
                pv = ptsA if (p - 1) % 2 == 0 else ptsB
                for h in (2 * (p - 1), 2 * (p - 1) + 1):
                    prev.append(lambda h=h, pv=pv: av_chain(pv, h, 0))
                    prev.append(lambda h=h, pv=pv: av_chain(pv, h, 1))
                    prev.append(lambda h=h: trans_block(h))
            if p < 8:
                pts = ptsA if p % 2 == 0 else ptsB
                pair = (2 * p, 2 * p + 1)
                for kc in range(8):
                    score_block(pts, pair, kc)
                    if kc < len(prev):
                        prev[kc]()
            else:
                for task in prev:
                    task()

        psDt.release()
        psDa.release()
        psDs.release()
        sbD.release()
        qkvp.release()

        # ---------------- Phase E: LN2 + transpose ---------------------------
        z2Tp = tc.alloc_tile_pool(name="z2T", bufs=1, side="right")
        z2T = z2Tp.tile([P, 8, WINDOW], BF16)
        # prefetch the first MLP weight block while LN2 runs
        wf1 = tc.alloc_tile_pool(name="wf1", bufs=1, side="right")
        w1r0 = wf1.tile([P, 8, 1024], BF16, tag="w1r", name="w1r0")
        for kc in range(8):
            nc.sync.dma_start(w1r0[:, kc, :], w1_d[kc * P:(kc + 1) * P, 0:1024])
        xz2 = tc.alloc_tile_pool(name="xz2", bufs=3, side="left")
        psE = tc.alloc_tile_pool(name="psE", bufs=3, space="PSUM")

        for t in range(8):
            at = attn[:, t, :]
            st = xz2.tile([P, 8], F32, tag="stats2")
            musum, mu, vsum = st[:, 0:1], st[:, 1:2], st[:, 2:3]
            veps, sdv, rstd = st[:, 4:5], st[:, 5:6], st[:, 6:7]
            nc.vector.reduce_sum(musum, at, axis=AX.X)
            nc.vector.tensor_scalar_mul(mu, musum, 1.0 / D)
            z = xz2.tile([P, D], BF16, tag="zE")
            zf = xz2.tile([P, D], F32, tag="zEf")
            nc.vector.scalar_tensor_tensor(
                zf[:], at, mu, at,
                op0=ALU.subtract, op1=ALU.mult, accum_out=vsum)
            nc.vector.tensor_scalar(veps, vsum, 1.0 / D, EPS,
                                    op0=ALU.mult, op1=ALU.add)
            nc.scalar.sqrt(sdv, veps)
            nc.vector.reciprocal(rstd, sdv)
            nc.vector.tensor_scalar(z[:], at, mu, rstd,
                                    op0=ALU.subtract, op1=ALU.mult)
            batch = psE.tile([P, D], BF16, tag="tpE")
            for c in range(8):
                nc.tensor.transpose(batch[:, c * P:(c + 1) * P],
                                    z[:, c * P:(c + 1) * P], identB[:])
            nc.scalar.copy(z2T[:, :, t * P:(t + 1) * P],
                           batch[:].rearrange("p (c n) -> p c n", n=P))

        psE.release()
        xz2.release()
        attnp.release()

        # ---------------- Phase F: MLP + residual ----------------------------
        h2p = tc.alloc_tile_pool(name="h2acc", bufs=1, side="left")
        h2acc = h2p.tile([P, 8, WINDOW], F32)
        xinTp = tc.alloc_tile_pool(name="xinT", bufs=1, side="left")
        xinT = xinTp.tile([P, 8, WINDOW], F32)
        nc.sync.dma_start(xinT[:], xinT_d.rearrange("(c p) n -> p c n", p=P))
        wf2 = tc.alloc_tile_pool(name="wf2", bufs=1, side="right")
        h1p = tc.alloc_tile_pool(name="h1p", bufs=1, side="left")
        tailp = tc.alloc_tile_pool(name="tail", bufs=3, side="left")
        psF1 = tc.alloc_tile_pool(name="psF1", bufs=4, space="PSUM")
        psF2 = tc.alloc_tile_pool(name="psF2", bufs=4, space="PSUM")

        for sc in range(4):
            if sc == 0:
                w1r = w1r0
            else:
                w1r = wf1.tile([P, 8, 1024], BF16, tag="w1r")
                for kc in range(8):
                    nc.sync.dma_start(
                        w1r[:, kc, :],
                        w1_d[kc * P:(kc + 1) * P, sc * 1024:(sc + 1) * 1024])
            h1 = h1p.tile([P, 8, WINDOW], BF16, tag="h1")
            # qh-outer: the first half only needs LN2 tiles 0-3, so the
            # MLP starts while the second half of LN2 is still running
            for qh in range(2):
                for ft in range(8):
                    hp = psF1.tile([P, 512], F32, tag="h1ps")
                    for kc in range(8):
                        nc.tensor.matmul(
                            hp[:], w1r[:, kc, ft * P:(ft + 1) * P],
                            z2T[:, kc, qh * 512:(qh + 1) * 512],
                            start=(kc == 0), stop=(kc == 7))
                    nc.scalar.activation(
                        h1[:, ft, qh * 512:(qh + 1) * 512], hp[:], AF.Silu,
                        bias=b1s[:, sc * 8 + ft:sc * 8 + ft + 1], scale=1.0)
            w2r = wf2.tile([P, 8, 1024], BF16, tag="w2r")
            for kc in range(8):
                nc.sync.dma_start(
                    w2r[:, kc, :],
                    w2_d[(sc * 8 + kc) * P:(sc * 8 + kc + 1) * P, :])
            for co in range(8):
                for qh in range(2):
                    hp2 = psF2.tile([P, 512], F32, tag="h2ps")
                    for kc in range(8):
                        nc.tensor.matmul(
                            hp2[:], w2r[:, kc, co * P:(co + 1) * P],
                            h1[:, kc, qh * 512:(qh + 1) * 512],
                            start=(kc == 0), stop=(kc == 7))
                    dstp = h2acc[:, co, qh * 512:(qh + 1) * 512]
                    if sc == 0:
                        nc.vector.tensor_copy(dstp, hp2[:])
                    elif sc < 3:
                        nc.vector.tensor_tensor(dstp, hp2[:], dstp,
                                                op=ALU.add)
                    else:
                        nc.vector.scalar_tensor_tensor(
                            dstp, hp2[:], b2s[:, co:co + 1], dstp,
                            op0=ALU.add, op1=ALU.add)
                if sc == 3:
                    # residual add + store as soon as this dim chunk is done
                    y = tailp.tile([P, WINDOW], F32, tag="y")
                    nc.vector.tensor_tensor(y[:], h2acc[:, co, :],
                                            xinT[:, co, :], op=ALU.add)
                    nc.sync.dma_start(y_d[co * P:(co + 1) * P, :], y[:])

        psF2.release()
        psF1.release()
        tailp.release()
        h1p.release()
        wf2.release()
        wf1.release()
        z2Tp.release()
        xinTp.release()
        h2p.release()
        cpool.release()

    nc.compile()
    return nc


def _prep_inputs(inputs):
    x = np.ascontiguousarray(np.asarray(inputs["x"], dtype=np.float32))
    kpm = np.asarray(inputs["key_pad_mask"]).astype(bool)
    wq = np.asarray(inputs["wq"], dtype=np.float32)
    wkv = np.asarray(inputs["wkv"], dtype=np.float32)
    w1 = np.asarray(inputs["w1"], dtype=np.float32)
    w2 = np.asarray(inputs["w2"], dtype=np.float32)
    bq = np.asarray(inputs["bq"], dtype=np.float32)
    bkv = np.asarray(inputs["bkv"], dtype=np.float32)
    b1 = np.asarray(inputs["b1"], dtype=np.float32)
    b2 = np.asarray(inputs["b2"], dtype=np.float32)
    ln1_g = np.asarray(inputs["ln1_g"], dtype=np.float32)
    ln1_b = np.asarray(inputs["ln1_b"], dtype=np.float32)
    ln2_g = np.asarray(inputs["ln2_g"], dtype=np.float32)
    ln2_b = np.asarray(inputs["ln2_b"], dtype=np.float32)

    # fold the LN affine transforms into the weights/biases (host-side):
    # (z*g + b) @ W + c == z @ (diag(g) W) + (b @ W + c)
    wq_f = ln1_g[:, None] * wq
    bq_f = ln1_b @ wq + bq
    wkv_f = ln1_g[:, None] * wkv
    bkv_f = ln1_b @ wkv + bkv
    w1_f = ln2_g[:, None] * w1
    b1_f = ln2_b @ w1 + b1

    def bf(v):
        return np.ascontiguousarray(v.astype(ml_dtypes.bfloat16))

    def dm(v):  # [D] -> [P, 8] dim-major chunk layout
        return np.ascontiguousarray(v.reshape(8, P).T)

    shared = {
        "wq": bf(wq_f),
        "wkv": bf(wkv_f),
        "w1": bf(w1_f),
        "w2": bf(w2),
        "bqs": np.ascontiguousarray((bq_f * ISD).reshape(8, P).T),
        "bkvk": dm(bkv_f[0:D]),
        "bkvvb": np.ascontiguousarray(
            np.broadcast_to(bkv_f[D:2 * D], (P, D)).astype(np.float32)),
        "b1s": np.ascontiguousarray(b1_f.reshape(32, P).T),
        "b2s": dm(b2),
    }

    ki = np.arange(P)[:, None]   # key index within block (partition/row)
    qi = np.arange(P)[None, :]   # query index within block (free/col)
    tri = np.where(ki > qi, np.float32(EXPMASK), np.float32(KEEPVAL))
    keep = np.full((P, P), np.float32(KEEPVAL), dtype=np.float32)
    full = np.full((P, P), np.float32(EXPMASK), dtype=np.float32)

    in_maps = []
    for core in range(8):
        b, h = core // 2, core % 2
        perm = [2 * s + h for s in range(8)]
        xq = np.ascontiguousarray(
            x[b, 0:WINDOW * 2].reshape(16, P, D)[perm].reshape(WINDOW, D))
        xw = x[b, S - WINDOW:S]
        pad = kpm[b, S - WINDOW:S]
        m = dict(shared)
        m["xin"] = np.ascontiguousarray(np.concatenate([xq, xw], axis=0))
        m["xinT"] = np.ascontiguousarray(xq.T)
        m["padb"] = np.ascontiguousarray(
            (MASKVAL * pad.astype(np.float32)).reshape(8, P).T)
        m["maskE"] = (tri if h == 0 else keep).astype(ml_dtypes.bfloat16)
        m["maskO"] = (full if h == 0 else tri).astype(ml_dtypes.bfloat16)
        in_maps.append(m)
    return in_maps


def kernel(**inputs):
    from concourse.bass_utils import run_bass_kernel_spmd

    if "nc" not in _CACHE:
        _CACHE["nc"] = _build_program()
    nc = _CACHE["nc"]

    in_maps = _prep_inputs(inputs)
    trace = os.environ.get("KERNEL_TRACE", "0") == "1"
    res = run_bass_kernel_spmd(nc, in_maps, core_ids=list(range(8)),
                               trace=trace)
    if res.exec_time_ns is not None:
        print(f"HW exec time: {res.exec_time_ns} ns")
        _CACHE["exec_time_ns"] = res.exec_time_ns
    out = np.empty((B, S, D), dtype=np.float32)
    for core in range(8):
        b, h = core // 2, core % 2
        yT = res.results[core]["y"].T.reshape(8, P, D)
        dst = out[b, 0:WINDOW * 2].reshape(16, P, D)
        for s in range(8):
            dst[2 * s + h] = yT[s]
    return out


# revision 21
# speedup vs baseline: 1.3924x; 1.0177x over previous
"""Trainium2 Bass kernel for a custom transformer block.

Sharding: 8 cores = 4 batches x 2 interleaved query-chunk sets. Core (b, h)
owns query chunks {2s+h : s in 0..8} (128 tokens each) of batch b; the KV
window (last 1024 tokens) is recomputed on both cores of a batch pair. The
stride-2 interleave balances the causal-triangular attention work across the
pair and lets the score matmuls skip fully-masked key/query blocks: the
score strip for key chunk kc only covers queries from slot kc//2.

All matmul operands are bf16 (fp32 PSUM accumulation); LN stats, softmax
denominators, the MLP accumulator and the residual stay fp32. LN gains and
biases are folded into the weight matrices and projection biases host-side,
so the LN transpose evacuations are batched plain copies (8 transposes into
one PSUM bank, one ACT copy out). Padding masks ride the exp evacuation as
a per-key ACT bias (-80 * pad); the causal diagonal needs a 2D mask only on
the first 128-query block of each key strip, applied post-exp on the bf16
probabilities by the otherwise-idle GpSimd engine (min with exp(-80)/BIG).
Attention is software-pipelined: pair p's score strips (scalar-exp-bound)
interleave with pair p-1's AV matmuls and transposes to keep the PE dense.
Scores are computed transposed ([key, query]); row sums come from an extra
ones-column on V and the normalization is a per-partition vector multiply
after the PE transpose back to token-major.
"""
import sys
import os

if "/opt/trn_rl_repo" not in sys.path:
    sys.path.insert(0, "/opt/trn_rl_repo")

import numpy as np
import ml_dtypes

B, S, D = 4, 2048, 1024
N_HEAD = 16
D_HEAD = 64
WINDOW = 1024
D_FF = 4096
EPS = 1e-5
ISD = float(1.0 / np.sqrt(D))  # 1/32
MASKVAL = -80.0
EXPMASK = float(np.exp(-80.0))  # 1.8e-35: effectively zero, bf16-normal
KEEPVAL = 3e38
P = 128

# first live query slot for key chunk kc (strip start = 128*S_MIN[kc]);
# slot s holds query chunk 2s+h, live when kc <= 2s+h -> s >= ceil((kc-1)/2)
S_MIN = [kc // 2 for kc in range(8)]  # == ceil((kc-1)/2): [0,0,1,1,2,2,3,3]

_CACHE = {}


def _build_program():
    import concourse.bacc as bacc
    import concourse.mybir as mybir
    from concourse.tile import TileContext
    from concourse.masks import make_identity

    F32 = mybir.dt.float32
    BF16 = mybir.dt.bfloat16
    AF = mybir.ActivationFunctionType
    ALU = mybir.AluOpType
    AX = mybir.AxisListType

    nc = bacc.Bacc("TRN2", target_bir_lowering=False, debug=False,
                   num_devices=8)

    xin_d = nc.dram_tensor("xin", [2 * WINDOW, D], BF16,
                           kind="ExternalInput")
    wq_d = nc.dram_tensor("wq", [D, D], BF16, kind="ExternalInput")
    wkv_d = nc.dram_tensor("wkv", [D, 2 * D], BF16, kind="ExternalInput")
    w1_d = nc.dram_tensor("w1", [D, D_FF], BF16, kind="ExternalInput")
    w2_d = nc.dram_tensor("w2", [D_FF, D], BF16, kind="ExternalInput")
    bqs_d = nc.dram_tensor("bqs", [P, 8], F32, kind="ExternalInput")
    bkvk_d = nc.dram_tensor("bkvk", [P, 8], F32, kind="ExternalInput")
    bkvvb_d = nc.dram_tensor("bkvvb", [P, D], F32, kind="ExternalInput")
    b1s_d = nc.dram_tensor("b1s", [P, 32], F32, kind="ExternalInput")
    b2s_d = nc.dram_tensor("b2s", [P, 8], F32, kind="ExternalInput")
    padb_d = nc.dram_tensor("padb", [P, 8], F32, kind="ExternalInput")
    maskE_d = nc.dram_tensor("maskE", [P, P], BF16, kind="ExternalInput")
    maskO_d = nc.dram_tensor("maskO", [P, P], BF16, kind="ExternalInput")
    xinT_d = nc.dram_tensor("xinT", [D, WINDOW], F32, kind="ExternalInput")
    y_d = nc.dram_tensor("y", [D, WINDOW], F32, kind="ExternalOutput")

    with TileContext(nc) as tc:
        cpool = tc.alloc_tile_pool(name="const", bufs=1, side="left")
        identB = cpool.tile([P, P], BF16)
        make_identity(nc, identB[:])
        masks = cpool.tile([P, 2 * P], BF16)
        maskE = masks[:, 0:P]
        maskO = masks[:, P:2 * P]
        nc.sync.dma_start(maskE, maskE_d[:])
        nc.sync.dma_start(maskO, maskO_d[:])
        smallc = cpool.tile([P, 80], F32)
        bqs = smallc[:, 0:8]
        bkvk = smallc[:, 8:16]
        b1s = smallc[:, 16:48]
        b2s = smallc[:, 48:56]
        onesc = smallc[:, 56:72]
        padb = smallc[:, 72:80]
        nc.vector.memset(onesc, 1.0)
        nc.sync.dma_start(bqs, bqs_d[:])
        nc.sync.dma_start(bkvk, bkvk_d[:])
        nc.sync.dma_start(b1s, b1s_d[:])
        nc.sync.dma_start(b2s, b2s_d[:])
        nc.sync.dma_start(padb, padb_d[:])

        # ---------------- Phase B/C: LN1 + QKV projections ------------------
        # z = (x - mu) * rstd token-major (LN gain/bias folded into weights);
        # 8 PE transposes batch into one PSUM bank, one ACT copy evacuates.
        # Window tiles (8-15) first so the V/K projections overlap the LN of
        # the query half, keeping the PE dense from the start.
        zTp = tc.alloc_tile_pool(name="zT", bufs=1, side="left")
        zqT = zTp.tile([P, 8, WINDOW], BF16)
        zwT = zTp.tile([P, 8, WINDOW], BF16)
        xz = tc.alloc_tile_pool(name="xz", bufs=3, side="left")
        psB = tc.alloc_tile_pool(name="psB", bufs=3, space="PSUM")

        def ln_stats(pool, xt, tag):
            """mean via DVE reduce, sum(x^2) via scalar Square+accumulator
            (runs in parallel); var = E[x^2] - mu^2."""
            st = pool.tile([P, 8], F32, tag="stats" + tag, name="st")
            junk = pool.tile([P, D], BF16, tag="junk" + tag, name="junk")
            musum, mu, sq = st[:, 0:1], st[:, 1:2], st[:, 2:3]
            mu2, veps, sdv, rstd = (st[:, 3:4], st[:, 4:5], st[:, 5:6],
                                    st[:, 6:7])
            nc.vector.reduce_sum(musum, xt, axis=AX.X)
            nc.scalar.activation(junk[:], xt, AF.Square, accum_out=sq)
            nc.vector.tensor_scalar_mul(mu, musum, 1.0 / D)
            nc.vector.tensor_tensor(mu2, mu, mu, op=ALU.mult)
            nc.vector.tensor_scalar(veps, sq, 1.0 / D, EPS,
                                    op0=ALU.mult, op1=ALU.add)
            nc.vector.tensor_tensor(veps, veps, mu2, op=ALU.subtract)
            nc.scalar.sqrt(sdv, veps)
            nc.vector.reciprocal(rstd, sdv)
            return mu, rstd

        def ln1_tile(t):
            xt = xz.tile([P, D], BF16, tag="x")
            nc.sync.dma_start(xt[:], xin_d[t * P:(t + 1) * P, :])
            mu, rstd = ln_stats(xz, xt[:], "1")
            z = xz.tile([P, D], BF16, tag="z")
            nc.vector.tensor_scalar(z[:], xt[:], mu, rstd,
                                    op0=ALU.subtract, op1=ALU.mult)
            batch = psB.tile([P, D], BF16, tag="tpB")
            for c in range(8):
                nc.tensor.transpose(batch[:, c * P:(c + 1) * P],
                                    z[:, c * P:(c + 1) * P], identB[:])
            dst = zqT if t < 8 else zwT
            col = (t % 8) * P
            nc.scalar.copy(dst[:, :, col:col + P],
                           batch[:].rearrange("p (c n) -> p c n", n=P))

        for t in range(8, 16):
            ln1_tile(t)

        qkvp = tc.alloc_tile_pool(name="qkv", bufs=1, side="right")
        qT = qkvp.tile([P, 8, WINDOW], BF16)      # q/sqrt(D), dim-major
        kT = qkvp.tile([P, 8, WINDOW], BF16)      # k, dim-major
        V = qkvp.tile([P, 8, N_HEAD * 65], BF16)  # token-major + ones col

        wst = tc.alloc_tile_pool(name="wst", bufs=1, side="left")
        psC = tc.alloc_tile_pool(name="psC", bufs=4, space="PSUM")

        # V: activations stationary -> token-major, bias added via bcast tile
        bkvvb = wst.tile([P, D], F32, tag="bkvvb")
        nc.sync.dma_start(bkvvb[:], bkvvb_d[:])

        def wload(w_d, c0):  # [D, 512] weight block, dim-chunked
            wr = wst.tile([P, 8, 512], BF16, tag="wkres", bufs=4, name="wr")
            for kc in range(8):
                nc.sync.dma_start(
                    wr[:, kc, :], w_d[kc * P:(kc + 1) * P, c0:c0 + 512])
            return wr

        def v_chain(tt, vh, wvr):
            pp = psC.tile([P, 512], F32, tag="proj")
            for kc in range(8):
                nc.tensor.matmul(
                    pp[:], zwT[:, kc, tt * P:(tt + 1) * P],
                    wvr[:, kc, :],
                    start=(kc == 0), stop=(kc == 7))
            vdst = V[:, tt, :].rearrange("p (h n) -> p h n", n=65)[
                :, vh * 8:(vh + 1) * 8, 0:64]
            nc.vector.scalar_tensor_tensor(
                vdst, pp[:].rearrange("p (h n) -> p h n", n=64), 0.0,
                bkvvb[:, vh * 512:(vh + 1) * 512].rearrange(
                    "p (h n) -> p h n", n=64),
                op0=ALU.add, op1=ALU.add)

        def kq_half(wkr, wh, qh, dst, src, bias, scale):
            for co in range(wh * 4, wh * 4 + 4):
                pp = psC.tile([P, 512], F32, tag="proj")
                for kc in range(8):
                    nc.tensor.matmul(
                        pp[:], wkr[:, kc, (co % 4) * P:(co % 4 + 1) * P],
                        src[:, kc, qh * 512:(qh + 1) * 512],
                        start=(kc == 0), stop=(kc == 7))
                nc.scalar.activation(
                    dst[:, co, qh * 512:(qh + 1) * 512], pp[:],
                    AF.Identity, bias=bias[:, co:co + 1], scale=scale)

        # window LN first; V token-chunk chains interleave with it, then
        # K (window-only) and Q interleave with the query-half LN
        ln1_tile(8)
        ln1_tile(9)
        wvr0 = wload(wkv_d, D)
        wvr1 = wload(wkv_d, D + 512)
        for tt in range(8):
            if tt + 10 < 16:
                ln1_tile(tt + 10)
            v_chain(tt, 0, wvr0)
            v_chain(tt, 1, wvr1)
        for tt in range(8):
            nc.scalar.copy(
                V[:, tt, :].rearrange("p (h n) -> p h n", n=65)[:, :, 64:65],
                onesc.rearrange("p (h n) -> p h n", n=1))
        wkr0 = wload(wkv_d, 0)
        wkr1 = wload(wkv_d, 512)
        ln1_tile(0)
        kq_half(wkr0, 0, 0, kT, zwT, bkvk, 1.0)
        ln1_tile(1)
        kq_half(wkr0, 0, 1, kT, zwT, bkvk, 1.0)
        ln1_tile(2)
        kq_half(wkr1, 1, 0, kT, zwT, bkvk, 1.0)
        ln1_tile(3)
        kq_half(wkr1, 1, 1, kT, zwT, bkvk, 1.0)
        wqr0 = wload(wq_d, 0)
        ln1_tile(4)
        kq_half(wqr0, 0, 0, qT, zqT, bqs, ISD)
        wqr1 = wload(wq_d, 512)
        ln1_tile(5)
        kq_half(wqr1, 1, 0, qT, zqT, bqs, ISD)
        ln1_tile(6)
        ln1_tile(7)
        kq_half(wqr0, 0, 1, qT, zqT, bqs, ISD)
        kq_half(wqr1, 1, 1, qT, zqT, bqs, ISD)

        psC.release()
        psB.release()
        wst.release()
        xz.release()
        zTp.release()

        # ---------------- Phase D: attention (software-pipelined) -----------
        attnp = tc.alloc_tile_pool(name="attn", bufs=1, side="left")
        attn = attnp.tile([P, 8, D], F32)          # normalized attn out
        rinva = attnp.tile([P, 8, N_HEAD], F32)    # per-head 1/rowsum
        # ping-pong exp(score) tiles: [pair-head, kc, query]; the columns
        # before each strip start are never written -> zero them once
        ptsA = attnp.tile([P, 2, 8, WINDOW], BF16)
        ptsB = attnp.tile([P, 2, 8, WINDOW], BF16)
        for pts in (ptsA, ptsB):
            for h2 in range(2):
                for kc in range(2, 8):
                    z0 = S_MIN[kc] * P
                    nc.vector.memset(pts[:, h2, kc, 0:z0], 0.0)

        sbD = tc.alloc_tile_pool(name="sbD", bufs=2, side="left")
        psDs = tc.alloc_tile_pool(name="psDs", bufs=2, space="PSUM")
        psDa = tc.alloc_tile_pool(name="psDa", bufs=2, space="PSUM")
        psDt = tc.alloc_tile_pool(name="psDt", bufs=2, space="PSUM")

        oa_tiles = {}

        def score_block(pts, pair, kc):
            z0 = S_MIN[kc] * P
            L = WINDOW - z0
            strips = {}
            for h in pair:
                strips[h] = psDs.tile([P, WINDOW], F32, tag="s", name="s")
            for c0, c1 in ((0, 512), (512, L)):
                for h in pair:
                    po, ch = (h % 2) * 64, h // 2
                    nc.tensor.matmul(
                        strips[h][:, c0:c1],
                        kT[po:po + 64, ch, kc * P:(kc + 1) * P],
                        qT[po:po + 64, ch, z0 + c0:z0 + c1],
                        start=True, stop=True)
            mk = maskE if kc % 2 == 0 else maskO
            for h in pair:
                nc.scalar.activation(
                    pts[:, h % 2, kc, z0:WINDOW], strips[h][:, 0:L],
                    AF.Exp, bias=padb[:, kc:kc + 1])
                # causal diagonal / dead block: min on the bf16 probs
                # (exp is monotone); all-SBUF 16-bit op -> 2x DVE rate
                nc.vector.tensor_tensor(
                    pts[:, h % 2, kc, z0:z0 + P],
                    pts[:, h % 2, kc, z0:z0 + P], mk, op=ALU.min)

        def av_chain(pts, h, qh):
            if qh == 0:
                oa_tiles[h] = sbD.tile([65, WINDOW], BF16, tag="oa",
                                       name="oa")
            avp = psDa.tile([65, 512], F32, tag="av")
            for kc in range(8):
                nc.tensor.matmul(
                    avp[:], V[:, kc, h * 65:(h + 1) * 65],
                    pts[:, h % 2, kc, qh * 512:(qh + 1) * 512],
                    start=(kc == 0), stop=(kc == 7))
            nc.vector.tensor_copy(oa_tiles[h][:, qh * 512:(qh + 1) * 512],
                                  avp[:])

        def trans_block(h, drain=False):
            oa = oa_tiles[h]
            # 66-wide groups keep each transpose output 4-byte aligned
            tpb = psDt.tile([P, 8, 66], BF16, tag="tp65")
            for t in range(8):
                nc.tensor.transpose(tpb[:, t, 0:65],
                                    oa[:, t * P:(t + 1) * P],
                                    identB[0:65, 0:65])
            for t in range(8):
                rinv = rinva[:, t, h:h + 1]
                nc.vector.reciprocal(rinv, tpb[:, t, 64:65])
                if drain:
                    # the drain's vector queue gates LN2; use idle scalar
                    nc.scalar.activation(
                        attn[:, t, h * 64:(h + 1) * 64], tpb[:, t, 0:64],
                        AF.Copy, scale=rinv)
                else:
                    nc.vector.tensor_scalar_mul(
                        attn[:, t, h * 64:(h + 1) * 64], tpb[:, t, 0:64],
                        rinv)

        for p in range(9):
            prev = []
            if p > 0:
                pv = ptsA if (p - 1) % 2 == 0 else ptsB
                for h in (2 * (p - 1), 2 * (p - 1) + 1):
                    prev.append(lambda h=h, pv=pv: av_chain(pv, h, 0))
                    prev.append(lambda h=h, pv=pv: av_chain(pv, h, 1))
                    prev.append(lambda h=h, dr=(p == 8): trans_block(h, dr))
            if p < 8:
                pts = ptsA if p % 2 == 0 else ptsB
                pair = (2 * p, 2 * p + 1)
                for kc in range(8):
                    score_block(pts, pair, kc)
                    if kc < len(prev):
                        prev[kc]()
            else:
                for task in prev:
                    task()

        psDt.release()
        psDa.release()
        psDs.release()
        sbD.release()
        qkvp.release()

        # ---------------- Phase E: LN2 + transpose ---------------------------
        z2Tp = tc.alloc_tile_pool(name="z2T", bufs=1, side="right")
        z2T = z2Tp.tile([P, 8, WINDOW], BF16)
        # prefetch the first MLP weight block while LN2 runs
        wf1 = tc.alloc_tile_pool(name="wf1", bufs=1, side="right")
        w1r0 = wf1.tile([P, 8, 1024], BF16, tag="w1r", name="w1r0")
        for kc in range(8):
            nc.sync.dma_start(w1r0[:, kc, :], w1_d[kc * P:(kc + 1) * P, 0:1024])
        xz2 = tc.alloc_tile_pool(name="xz2", bufs=3, side="left")
        psE = tc.alloc_tile_pool(name="psE", bufs=3, space="PSUM")

        for t in range(8):
            at = attn[:, t, :]
            mu, rstd = ln_stats(xz2, at, "2")
            z = xz2.tile([P, D], BF16, tag="zE")
            nc.vector.tensor_scalar(z[:], at, mu, rstd,
                                    op0=ALU.subtract, op1=ALU.mult)
            batch = psE.tile([P, D], BF16, tag="tpE")
            for c in range(8):
                nc.tensor.transpose(batch[:, c * P:(c + 1) * P],
                                    z[:, c * P:(c + 1) * P], identB[:])
            nc.scalar.copy(z2T[:, :, t * P:(t + 1) * P],
                           batch[:].rearrange("p (c n) -> p c n", n=P))

        psE.release()
        xz2.release()
        attnp.release()

        # ---------------- Phase F: MLP + residual ----------------------------
        h2p = tc.alloc_tile_pool(name="h2acc", bufs=1, side="left")
        h2acc = h2p.tile([P, 8, WINDOW], F32)
        xinTp = tc.alloc_tile_pool(name="xinT", bufs=1, side="left")
        xinT = xinTp.tile([P, 8, WINDOW], F32)
        nc.sync.dma_start(xinT[:], xinT_d.rearrange("(c p) n -> p c n", p=P))
        wf2 = tc.alloc_tile_pool(name="wf2", bufs=1, side="right")
        h1p = tc.alloc_tile_pool(name="h1p", bufs=1, side="left")
        tailp = tc.alloc_tile_pool(name="tail", bufs=3, side="left")
        psF1 = tc.alloc_tile_pool(name="psF1", bufs=4, space="PSUM")
        psF2 = tc.alloc_tile_pool(name="psF2", bufs=4, space="PSUM")

        for sc in range(4):
            if sc == 0:
                w1r = w1r0
            else:
                w1r = wf1.tile([P, 8, 1024], BF16, tag="w1r")
                for kc in range(8):
                    nc.sync.dma_start(
                        w1r[:, kc, :],
                        w1_d[kc * P:(kc + 1) * P, sc * 1024:(sc + 1) * 1024])
            h1 = h1p.tile([P, 8, WINDOW], BF16, tag="h1")
            # qh-outer: the first half only needs LN2 tiles 0-3, so the
            # MLP starts while the second half of LN2 is still running
            for qh in range(2):
                for ft in range(8):
                    hp = psF1.tile([P, 512], F32, tag="h1ps")
                    for kc in range(8):
                        nc.tensor.matmul(
                            hp[:], w1r[:, kc, ft * P:(ft + 1) * P],
                            z2T[:, kc, qh * 512:(qh + 1) * 512],
                            start=(kc == 0), stop=(kc == 7))
                    nc.scalar.activation(
                        h1[:, ft, qh * 512:(qh + 1) * 512], hp[:], AF.Silu,
                        bias=b1s[:, sc * 8 + ft:sc * 8 + ft + 1], scale=1.0)
            w2r = wf2.tile([P, 8, 1024], BF16, tag="w2r")
            for kc in range(8):
                nc.sync.dma_start(
                    w2r[:, kc, :],
                    w2_d[(sc * 8 + kc) * P:(sc * 8 + kc + 1) * P, :])
            for co in range(8):
                for qh in range(2):
                    hp2 = psF2.tile([P, 512], F32, tag="h2ps")
                    for kc in range(8):
                        nc.tensor.matmul(
                            hp2[:], w2r[:, kc, co * P:(co + 1) * P],
                            h1[:, kc, qh * 512:(qh + 1) * 512],
                            start=(kc == 0), stop=(kc == 7))
                    dstp = h2acc[:, co, qh * 512:(qh + 1) * 512]
                    if sc == 0:
                        nc.vector.tensor_copy(dstp, hp2[:])
                    elif sc < 3:
                        nc.vector.tensor_tensor(dstp, hp2[:], dstp,
                                                op=ALU.add)
                    else:
                        nc.vector.scalar_tensor_tensor(
                            dstp, hp2[:], b2s[:, co:co + 1], dstp,
                            op0=ALU.add, op1=ALU.add)
                if sc == 3:
                    # residual add + store as soon as this dim chunk is done
                    y = tailp.tile([P, WINDOW], F32, tag="y")
                    nc.vector.tensor_tensor(y[:], h2acc[:, co, :],
                                            xinT[:, co, :], op=ALU.add)
                    nc.sync.dma_start(y_d[co * P:(co + 1) * P, :], y[:])

        psF2.release()
        psF1.release()
        tailp.release()
        h1p.release()
        wf2.release()
        wf1.release()
        z2Tp.release()
        xinTp.release()
        h2p.release()
        cpool.release()

    nc.compile()
    return nc


def _prep_inputs(inputs):
    x = np.ascontiguousarray(np.asarray(inputs["x"], dtype=np.float32))
    kpm = np.asarray(inputs["key_pad_mask"]).astype(bool)
    wq = np.asarray(inputs["wq"], dtype=np.float32)
    wkv = np.asarray(inputs["wkv"], dtype=np.float32)
    w1 = np.asarray(inputs["w1"], dtype=np.float32)
    w2 = np.asarray(inputs["w2"], dtype=np.float32)
    bq = np.asarray(inputs["bq"], dtype=np.float32)
    bkv = np.asarray(inputs["bkv"], dtype=np.float32)
    b1 = np.asarray(inputs["b1"], dtype=np.float32)
    b2 = np.asarray(inputs["b2"], dtype=np.float32)
    ln1_g = np.asarray(inputs["ln1_g"], dtype=np.float32)
    ln1_b = np.asarray(inputs["ln1_b"], dtype=np.float32)
    ln2_g = np.asarray(inputs["ln2_g"], dtype=np.float32)
    ln2_b = np.asarray(inputs["ln2_b"], dtype=np.float32)

    # fold the LN affine transforms into the weights/biases (host-side):
    # (z*g + b) @ W + c == z @ (diag(g) W) + (b @ W + c)
    wq_f = ln1_g[:, None] * wq
    bq_f = ln1_b @ wq + bq
    wkv_f = ln1_g[:, None] * wkv
    bkv_f = ln1_b @ wkv + bkv
    w1_f = ln2_g[:, None] * w1
    b1_f = ln2_b @ w1 + b1

    def bf(v):
        return np.ascontiguousarray(v.astype(ml_dtypes.bfloat16))

    def dm(v):  # [D] -> [P, 8] dim-major chunk layout
        return np.ascontiguousarray(v.reshape(8, P).T)

    shared = {
        "wq": bf(wq_f),
        "wkv": bf(wkv_f),
        "w1": bf(w1_f),
        "w2": bf(w2),
        "bqs": np.ascontiguousarray((bq_f * ISD).reshape(8, P).T),
        "bkvk": dm(bkv_f[0:D]),
        "bkvvb": np.ascontiguousarray(
            np.broadcast_to(bkv_f[D:2 * D], (P, D)).astype(np.float32)),
        "b1s": np.ascontiguousarray(b1_f.reshape(32, P).T),
        "b2s": dm(b2),
    }

    ki = np.arange(P)[:, None]   # key index within block (partition/row)
    qi = np.arange(P)[None, :]   # query index within block (free/col)
    tri = np.where(ki > qi, np.float32(EXPMASK), np.float32(KEEPVAL))
    keep = np.full((P, P), np.float32(KEEPVAL), dtype=np.float32)
    full = np.full((P, P), np.float32(EXPMASK), dtype=np.float32)

    in_maps = []
    for core in range(8):
        b, h = core // 2, core % 2
        perm = [2 * s + h for s in range(8)]
        xq = np.ascontiguousarray(
            x[b, 0:WINDOW * 2].reshape(16, P, D)[perm].reshape(WINDOW, D))
        xw = x[b, S - WINDOW:S]
        pad = kpm[b, S - WINDOW:S]
        m = dict(shared)
        m["xin"] = np.ascontiguousarray(
            np.concatenate([xq, xw], axis=0).astype(ml_dtypes.bfloat16))
        m["xinT"] = np.ascontiguousarray(xq.T)
        m["padb"] = np.ascontiguousarray(
            (MASKVAL * pad.astype(np.float32)).reshape(8, P).T)
        m["maskE"] = (tri if h == 0 else keep).astype(ml_dtypes.bfloat16)
        m["maskO"] = (full if h == 0 else tri).astype(ml_dtypes.bfloat16)
        in_maps.append(m)
    return in_maps


def kernel(**inputs):
    from concourse.bass_utils import run_bass_kernel_spmd

    if "nc" not in _CACHE:
        _CACHE["nc"] = _build_program()
    nc = _CACHE["nc"]

    in_maps = _prep_inputs(inputs)
    trace = os.environ.get("KERNEL_TRACE", "0") == "1"
    res = run_bass_kernel_spmd(nc, in_maps, core_ids=list(range(8)),
                               trace=trace)
    if res.exec_time_ns is not None:
        print(f"HW exec time: {res.exec_time_ns} ns")
        _CACHE["exec_time_ns"] = res.exec_time_ns
    out = np.empty((B, S, D), dtype=np.float32)
    for core in range(8):
        b, h = core // 2, core % 2
        yT = res.results[core]["y"].T.reshape(8, P, D)
        dst = out[b, 0:WINDOW * 2].reshape(16, P, D)
        for s in range(8):
            dst[2 * s + h] = yT[s]
    return out


# revision 30
# speedup vs baseline: 1.3983x; 1.0043x over previous
"""Trainium2 Bass kernel for a custom transformer block.

Sharding: 8 cores = 4 batches x 2 interleaved query-chunk sets. Core (b, h)
owns query chunks {2s+h : s in 0..8} (128 tokens each) of batch b; the KV
window (last 1024 tokens) is recomputed on both cores of a batch pair. The
stride-2 interleave balances the causal-triangular attention work across the
pair and lets the score matmuls skip fully-masked key/query blocks: the
score strip for key chunk kc only covers queries from slot kc//2.

All matmul operands are bf16 (fp32 PSUM accumulation); LN stats, softmax
denominators, the MLP accumulator and the residual stay fp32. LN gains and
biases are folded into the weight matrices and projection biases host-side,
so the LN transpose evacuations are batched plain copies (8 transposes into
one PSUM bank, one ACT copy out). Padding masks ride the exp evacuation as
a per-key ACT bias (-80 * pad); the causal diagonal needs a 2D mask only on
the first 128-query block of each key strip, applied post-exp on the bf16
probabilities by the otherwise-idle GpSimd engine (min with exp(-80)/BIG).
Attention is software-pipelined: pair p's score strips (scalar-exp-bound)
interleave with pair p-1's AV matmuls and transposes to keep the PE dense.
Scores are computed transposed ([key, query]); row sums come from an extra
ones-column on V and the normalization is a per-partition vector multiply
after the PE transpose back to token-major.
"""
import sys
import os

if "/opt/trn_rl_repo" not in sys.path:
    sys.path.insert(0, "/opt/trn_rl_repo")

import numpy as np
import ml_dtypes

B, S, D = 4, 2048, 1024
N_HEAD = 16
D_HEAD = 64
WINDOW = 1024
D_FF = 4096
EPS = 1e-5
ISD = float(1.0 / np.sqrt(D))  # 1/32
MASKVAL = -80.0
EXPMASK = float(np.exp(-80.0))  # 1.8e-35: effectively zero, bf16-normal
KEEPVAL = 3e38
P = 128

# first live query slot for key chunk kc (strip start = 128*S_MIN[kc]);
# slot s holds query chunk 2s+h, live when kc <= 2s+h -> s >= ceil((kc-1)/2)
S_MIN = [kc // 2 for kc in range(8)]  # == ceil((kc-1)/2): [0,0,1,1,2,2,3,3]

_CACHE = {}


def _build_program():
    import concourse.bacc as bacc
    import concourse.mybir as mybir
    from concourse.tile import TileContext
    from concourse.masks import make_identity

    F32 = mybir.dt.float32
    BF16 = mybir.dt.bfloat16
    AF = mybir.ActivationFunctionType
    ALU = mybir.AluOpType
    AX = mybir.AxisListType

    nc = bacc.Bacc("TRN2", target_bir_lowering=False, debug=False,
                   num_devices=8)

    xin_d = nc.dram_tensor("xin", [2 * WINDOW, D], BF16,
                           kind="ExternalInput")
    wq_d = nc.dram_tensor("wq", [D, D], BF16, kind="ExternalInput")
    wkv_d = nc.dram_tensor("wkv", [D, 2 * D], BF16, kind="ExternalInput")
    w1_d = nc.dram_tensor("w1", [D, D_FF], BF16, kind="ExternalInput")
    w2_d = nc.dram_tensor("w2", [D_FF, D], BF16, kind="ExternalInput")
    # all small per-partition constants ride in one DMA:
    # [bqs 0:8 | bkvk 8:16 | b1s 16:48 | b2s 48:56 | padb 56:64]
    consts_d = nc.dram_tensor("consts", [P, 64], F32, kind="ExternalInput")
    bkvvb_d = nc.dram_tensor("bkvvb", [P, D], F32, kind="ExternalInput")
    masks_d = nc.dram_tensor("masks", [P, 2 * P], BF16, kind="ExternalInput")
    xinT_d = nc.dram_tensor("xinT", [D, WINDOW], F32, kind="ExternalInput")
    y_d = nc.dram_tensor("y", [D, WINDOW], F32, kind="ExternalOutput")

    with TileContext(nc) as tc:
        cpool = tc.alloc_tile_pool(name="const", bufs=1, side="left")
        identB = cpool.tile([P, P], BF16)
        make_identity(nc, identB[:])
        masks = cpool.tile([P, 2 * P], BF16)
        maskE = masks[:, 0:P]
        maskO = masks[:, P:2 * P]
        smallc = cpool.tile([P, 80], F32)
        bqs = smallc[:, 0:8]
        bkvk = smallc[:, 8:16]
        b1s = smallc[:, 16:48]
        b2s = smallc[:, 48:56]
        padb = smallc[:, 56:64]
        onesc = smallc[:, 64:80]
        nc.vector.memset(onesc, 1.0)

        def load_consts():  # deferred so the x DMAs win the queue
            nc.sync.dma_start(smallc[:, 0:64], consts_d[:])
            nc.sync.dma_start(masks[:], masks_d[:])

        # ---------------- Phase B/C: LN1 + QKV projections ------------------
        # z = (x - mu) * rstd token-major (LN gain/bias folded into weights);
        # 8 PE transposes batch into one PSUM bank, one ACT copy evacuates.
        # Window tiles (8-15) first so the V/K projections overlap the LN of
        # the query half, keeping the PE dense from the start.
        zTp = tc.alloc_tile_pool(name="zT", bufs=1, side="left")
        zqT = zTp.tile([P, 8, WINDOW], BF16)
        zwT = zTp.tile([P, 8, WINDOW], BF16)
        xz = tc.alloc_tile_pool(name="xz", bufs=3, side="left")
        psB = tc.alloc_tile_pool(name="psB", bufs=3, space="PSUM")

        def ln_stats(pool, xt, tag):
            """mean via DVE reduce, sum(x^2) via scalar Square+accumulator
            (runs in parallel); var = E[x^2] - mu^2."""
            st = pool.tile([P, 8], F32, tag="stats" + tag, name="st")
            junk = pool.tile([P, D], BF16, tag="junk" + tag, name="junk")
            musum, mu, sq = st[:, 0:1], st[:, 1:2], st[:, 2:3]
            mu2, veps, sdv, rstd = (st[:, 3:4], st[:, 4:5], st[:, 5:6],
                                    st[:, 6:7])
            nc.vector.reduce_sum(musum, xt, axis=AX.X)
            nc.scalar.activation(junk[:], xt, AF.Square, accum_out=sq)
            nc.vector.tensor_scalar_mul(mu, musum, 1.0 / D)
            nc.vector.tensor_tensor(mu2, mu, mu, op=ALU.mult)
            nc.vector.tensor_scalar(veps, sq, 1.0 / D, EPS,
                                    op0=ALU.mult, op1=ALU.add)
            nc.vector.tensor_tensor(veps, veps, mu2, op=ALU.subtract)
            nc.scalar.sqrt(sdv, veps)
            nc.vector.reciprocal(rstd, sdv)
            return mu, rstd

        def ln1_tile(t):
            xt = xz.tile([P, D], BF16, tag="x")
            nc.sync.dma_start(xt[:], xin_d[t * P:(t + 1) * P, :])
            mu, rstd = ln_stats(xz, xt[:], "1")
            z = xz.tile([P, D], BF16, tag="z")
            nc.vector.tensor_scalar(z[:], xt[:], mu, rstd,
                                    op0=ALU.subtract, op1=ALU.mult)
            batch = psB.tile([P, D], BF16, tag="tpB")
            for c in range(8):
                nc.tensor.transpose(batch[:, c * P:(c + 1) * P],
                                    z[:, c * P:(c + 1) * P], identB[:])
            dst = zqT if t < 8 else zwT
            col = (t % 8) * P
            nc.scalar.copy(dst[:, :, col:col + P],
                           batch[:].rearrange("p (c n) -> p c n", n=P))

        for t in range(8, 16):
            ln1_tile(t)

        qkvp = tc.alloc_tile_pool(name="qkv", bufs=1, side="right")
        qT = qkvp.tile([P, 8, WINDOW], BF16)      # q/sqrt(D), dim-major
        kT = qkvp.tile([P, 8, WINDOW], BF16)      # k, dim-major
        V = qkvp.tile([P, 8, N_HEAD * 65], BF16)  # token-major + ones col

        wst = tc.alloc_tile_pool(name="wst", bufs=1, side="left")
        psC = tc.alloc_tile_pool(name="psC", bufs=3, space="PSUM")

        # V: activations stationary -> token-major, bias added via bcast tile
        bkvvb = wst.tile([P, D], F32, tag="bkvvb")
        nc.sync.dma_start(bkvvb[:], bkvvb_d[:])

        def wload(w_d, c0):  # [D, 512] weight block, dim-chunked
            wr = wst.tile([P, 8, 512], BF16, tag="wkres", bufs=4, name="wr")
            for kc in range(8):
                nc.sync.dma_start(
                    wr[:, kc, :], w_d[kc * P:(kc + 1) * P, c0:c0 + 512])
            return wr

        def v_chain(tt, vh, wvr):
            pp = psC.tile([P, 512], F32, tag="proj")
            for kc in range(8):
                nc.tensor.matmul(
                    pp[:], zwT[:, kc, tt * P:(tt + 1) * P],
                    wvr[:, kc, :],
                    start=(kc == 0), stop=(kc == 7))
            vdst = V[:, tt, :].rearrange("p (h n) -> p h n", n=65)[
                :, vh * 8:(vh + 1) * 8, 0:64]
            nc.vector.scalar_tensor_tensor(
                vdst, pp[:].rearrange("p (h n) -> p h n", n=64), 0.0,
                bkvvb[:, vh * 512:(vh + 1) * 512].rearrange(
                    "p (h n) -> p h n", n=64),
                op0=ALU.add, op1=ALU.add)

        def kq_half(wkr, wh, qh, dst, src, bias, scale):
            for co in range(wh * 4, wh * 4 + 4):
                pp = psC.tile([P, 512], F32, tag="proj")
                for kc in range(8):
                    nc.tensor.matmul(
                        pp[:], wkr[:, kc, (co % 4) * P:(co % 4 + 1) * P],
                        src[:, kc, qh * 512:(qh + 1) * 512],
                        start=(kc == 0), stop=(kc == 7))
                nc.scalar.activation(
                    dst[:, co, qh * 512:(qh + 1) * 512], pp[:],
                    AF.Identity, bias=bias[:, co:co + 1], scale=scale)

        # window LN first; V token-chunk chains interleave with it, then
        # K (window-only) and Q interleave with the query-half LN
        ln1_tile(8)
        ln1_tile(9)
        load_consts()
        wvr0 = wload(wkv_d, D)
        wvr1 = wload(wkv_d, D + 512)
        for tt in range(8):
            if tt + 10 < 16:
                ln1_tile(tt + 10)
            v_chain(tt, 0, wvr0)
            v_chain(tt, 1, wvr1)
        for tt in range(8):
            nc.scalar.copy(
                V[:, tt, :].rearrange("p (h n) -> p h n", n=65)[:, :, 64:65],
                onesc.rearrange("p (h n) -> p h n", n=1))
        wkr0 = wload(wkv_d, 0)
        wkr1 = wload(wkv_d, 512)
        ln1_tile(0)
        kq_half(wkr0, 0, 0, kT, zwT, bkvk, 1.0)
        ln1_tile(1)
        kq_half(wkr0, 0, 1, kT, zwT, bkvk, 1.0)
        ln1_tile(2)
        kq_half(wkr1, 1, 0, kT, zwT, bkvk, 1.0)
        ln1_tile(3)
        kq_half(wkr1, 1, 1, kT, zwT, bkvk, 1.0)
        wqr0 = wload(wq_d, 0)
        ln1_tile(4)
        kq_half(wqr0, 0, 0, qT, zqT, bqs, ISD)
        wqr1 = wload(wq_d, 512)
        ln1_tile(5)
        kq_half(wqr1, 1, 0, qT, zqT, bqs, ISD)
        ln1_tile(6)
        ln1_tile(7)
        kq_half(wqr0, 0, 1, qT, zqT, bqs, ISD)
        kq_half(wqr1, 1, 1, qT, zqT, bqs, ISD)

        # psC at 3 bufs leaves the score pool mostly on psB's banks, whose
        # tiles die before the last Q evacuations (less phase-handoff stall)
        psC.release()
        psB.release()
        wst.release()
        xz.release()
        zTp.release()

        # ---------------- Phase D: attention (software-pipelined) -----------
        attnp = tc.alloc_tile_pool(name="attn", bufs=1, side="left")
        attn = attnp.tile([P, 8, D], F32)          # normalized attn out
        rinva = attnp.tile([P, 8, N_HEAD], F32)    # per-head 1/rowsum
        # ping-pong exp(score) tiles: [pair-head, kc, query]; the columns
        # before each strip start are never written -> zero them once
        ptsA = attnp.tile([P, 2, 8, WINDOW], BF16)
        ptsB = attnp.tile([P, 2, 8, WINDOW], BF16)
        for pts in (ptsA, ptsB):
            for h2 in range(2):
                for kc in range(2, 8):
                    z0 = S_MIN[kc] * P
                    nc.vector.memset(pts[:, h2, kc, 0:z0], 0.0)

        sbD = tc.alloc_tile_pool(name="sbD", bufs=2, side="left")
        psDs = tc.alloc_tile_pool(name="psDs", bufs=2, space="PSUM")
        psDa = tc.alloc_tile_pool(name="psDa", bufs=2, space="PSUM")
        psDt = tc.alloc_tile_pool(name="psDt", bufs=2, space="PSUM")

        oa_tiles = {}

        def score_block(pts, pair, kc):
            z0 = S_MIN[kc] * P
            L = WINDOW - z0
            strips = {}
            for h in pair:
                strips[h] = psDs.tile([P, WINDOW], F32, tag="s", name="s")
            for c0, c1 in ((0, 512), (512, L)):
                for h in pair:
                    po, ch = (h % 2) * 64, h // 2
                    nc.tensor.matmul(
                        strips[h][:, c0:c1],
                        kT[po:po + 64, ch, kc * P:(kc + 1) * P],
                        qT[po:po + 64, ch, z0 + c0:z0 + c1],
                        start=True, stop=True)
            mk = maskE if kc % 2 == 0 else maskO
            for h in pair:
                nc.scalar.activation(
                    pts[:, h % 2, kc, z0:WINDOW], strips[h][:, 0:L],
                    AF.Exp, bias=padb[:, kc:kc + 1])
                # causal diagonal / dead block: min on the bf16 probs
                # (exp is monotone); all-SBUF 16-bit op -> 2x DVE rate
                nc.vector.tensor_tensor(
                    pts[:, h % 2, kc, z0:z0 + P],
                    pts[:, h % 2, kc, z0:z0 + P], mk, op=ALU.min)

        def av_chain(pts, h, qh):
            if qh == 0:
                oa_tiles[h] = sbD.tile([65, WINDOW], BF16, tag="oa",
                                       name="oa")
            avp = psDa.tile([65, 512], F32, tag="av")
            for kc in range(8):
                nc.tensor.matmul(
                    avp[:], V[:, kc, h * 65:(h + 1) * 65],
                    pts[:, h % 2, kc, qh * 512:(qh + 1) * 512],
                    start=(kc == 0), stop=(kc == 7))
            nc.vector.tensor_copy(oa_tiles[h][:, qh * 512:(qh + 1) * 512],
                                  avp[:])

        def trans_block(h, drain=False):
            oa = oa_tiles[h]
            # 66-wide groups keep each transpose output 4-byte aligned
            tpb = psDt.tile([P, 8, 66], BF16, tag="tp65")
            for t in range(8):
                nc.tensor.transpose(tpb[:, t, 0:65],
                                    oa[:, t * P:(t + 1) * P],
                                    identB[0:65, 0:65])
            for t in range(8):
                rinv = rinva[:, t, h:h + 1]
                nc.vector.reciprocal(rinv, tpb[:, t, 64:65])
                if drain:
                    # the drain's vector queue gates LN2; use idle scalar
                    nc.scalar.activation(
                        attn[:, t, h * 64:(h + 1) * 64], tpb[:, t, 0:64],
                        AF.Copy, scale=rinv)
                else:
                    nc.vector.tensor_scalar_mul(
                        attn[:, t, h * 64:(h + 1) * 64], tpb[:, t, 0:64],
                        rinv)

        for p in range(9):
            prev = []
            if p > 0:
                pv = ptsA if (p - 1) % 2 == 0 else ptsB
                for h in (2 * (p - 1), 2 * (p - 1) + 1):
                    prev.append(lambda h=h, pv=pv: av_chain(pv, h, 0))
                    prev.append(lambda h=h, pv=pv: av_chain(pv, h, 1))
                    prev.append(lambda h=h, dr=(p == 8 and h % 2 == 1):
                                trans_block(h, dr))
            if p < 8:
                pts = ptsA if p % 2 == 0 else ptsB
                pair = (2 * p, 2 * p + 1)
                for kc in range(8):
                    score_block(pts, pair, kc)
                    if kc < len(prev):
                        prev[kc]()
            else:
                for task in prev:
                    task()

        psDt.release()
        psDa.release()
        psDs.release()
        sbD.release()
        qkvp.release()

        # ---------------- Phase E: LN2 + transpose ---------------------------
        z2Tp = tc.alloc_tile_pool(name="z2T", bufs=1, side="right")
        z2T = z2Tp.tile([P, 8, WINDOW], BF16)
        # prefetch the first MLP weight block while LN2 runs
        wf1 = tc.alloc_tile_pool(name="wf1", bufs=1, side="right")
        w1r0 = wf1.tile([P, 8, 1024], BF16, tag="w1r", name="w1r0")
        for kc in range(8):
            nc.sync.dma_start(w1r0[:, kc, :], w1_d[kc * P:(kc + 1) * P, 0:1024])
        xz2 = tc.alloc_tile_pool(name="xz2", bufs=3, side="left")
        psE = tc.alloc_tile_pool(name="psE", bufs=3, space="PSUM")

        for t in range(8):
            at = attn[:, t, :]
            mu, rstd = ln_stats(xz2, at, "2")
            z = xz2.tile([P, D], BF16, tag="zE")
            nc.vector.tensor_scalar(z[:], at, mu, rstd,
                                    op0=ALU.subtract, op1=ALU.mult)
            batch = psE.tile([P, D], BF16, tag="tpE")
            for c in range(8):
                nc.tensor.transpose(batch[:, c * P:(c + 1) * P],
                                    z[:, c * P:(c + 1) * P], identB[:])
            nc.scalar.copy(z2T[:, :, t * P:(t + 1) * P],
                           batch[:].rearrange("p (c n) -> p c n", n=P))

        psE.release()
        xz2.release()
        attnp.release()

        # ---------------- Phase F: MLP + residual ----------------------------
        h2p = tc.alloc_tile_pool(name="h2acc", bufs=1, side="left")
        h2acc = h2p.tile([P, 8, WINDOW], F32)
        xinTp = tc.alloc_tile_pool(name="xinT", bufs=1, side="left")
        xinT = xinTp.tile([P, 8, WINDOW], F32)
        nc.sync.dma_start(xinT[:], xinT_d.rearrange("(c p) n -> p c n", p=P))
        wf2 = tc.alloc_tile_pool(name="wf2", bufs=1, side="right")
        h1p = tc.alloc_tile_pool(name="h1p", bufs=1, side="left")
        tailp = tc.alloc_tile_pool(name="tail", bufs=3, side="left")
        psF1 = tc.alloc_tile_pool(name="psF1", bufs=4, space="PSUM")
        psF2 = tc.alloc_tile_pool(name="psF2", bufs=4, space="PSUM")

        for sc in range(4):
            if sc == 0:
                w1r = w1r0
            else:
                w1r = wf1.tile([P, 8, 1024], BF16, tag="w1r")
                for kc in range(8):
                    nc.sync.dma_start(
                        w1r[:, kc, :],
                        w1_d[kc * P:(kc + 1) * P, sc * 1024:(sc + 1) * 1024])
            h1 = h1p.tile([P, 8, WINDOW], BF16, tag="h1")
            # qh-outer: the first half only needs LN2 tiles 0-3, so the
            # MLP starts while the second half of LN2 is still running
            for qh in range(2):
                for ft in range(8):
                    hp = psF1.tile([P, 512], F32, tag="h1ps")
                    for kc in range(8):
                        nc.tensor.matmul(
                            hp[:], w1r[:, kc, ft * P:(ft + 1) * P],
                            z2T[:, kc, qh * 512:(qh + 1) * 512],
                            start=(kc == 0), stop=(kc == 7))
                    nc.scalar.activation(
                        h1[:, ft, qh * 512:(qh + 1) * 512], hp[:], AF.Silu,
                        bias=b1s[:, sc * 8 + ft:sc * 8 + ft + 1], scale=1.0)
            w2r = wf2.tile([P, 8, 1024], BF16, tag="w2r")
            for kc in range(8):
                nc.sync.dma_start(
                    w2r[:, kc, :],
                    w2_d[(sc * 8 + kc) * P:(sc * 8 + kc + 1) * P, :])
            for co in range(8):
                for qh in range(2):
                    hp2 = psF2.tile([P, 512], F32, tag="h2ps")
                    for kc in range(8):
                        nc.tensor.matmul(
                            hp2[:], w2r[:, kc, co * P:(co + 1) * P],
                            h1[:, kc, qh * 512:(qh + 1) * 512],
                            start=(kc == 0), stop=(kc == 7))
                    dstp = h2acc[:, co, qh * 512:(qh + 1) * 512]
                    if sc == 0:
                        nc.vector.tensor_copy(dstp, hp2[:])
                    elif sc < 3:
                        nc.vector.tensor_tensor(dstp, hp2[:], dstp,
                                                op=ALU.add)
                    else:
                        nc.vector.scalar_tensor_tensor(
                            dstp, hp2[:], b2s[:, co:co + 1], dstp,
                            op0=ALU.add, op1=ALU.add)
                if sc == 3:
                    # residual add + store as soon as this dim chunk is done
                    y = tailp.tile([P, WINDOW], F32, tag="y")
                    nc.vector.tensor_tensor(y[:], h2acc[:, co, :],
                                            xinT[:, co, :], op=ALU.add)
                    nc.sync.dma_start(y_d[co * P:(co + 1) * P, :], y[:])

        psF2.release()
        psF1.release()
        tailp.release()
        h1p.release()
        wf2.release()
        wf1.release()
        z2Tp.release()
        xinTp.release()
        h2p.release()
        cpool.release()

    nc.compile()
    return nc


def _prep_inputs(inputs):
    x = np.ascontiguousarray(np.asarray(inputs["x"], dtype=np.float32))
    kpm = np.asarray(inputs["key_pad_mask"]).astype(bool)
    wq = np.asarray(inputs["wq"], dtype=np.float32)
    wkv = np.asarray(inputs["wkv"], dtype=np.float32)
    w1 = np.asarray(inputs["w1"], dtype=np.float32)
    w2 = np.asarray(inputs["w2"], dtype=np.float32)
    bq = np.asarray(inputs["bq"], dtype=np.float32)
    bkv = np.asarray(inputs["bkv"], dtype=np.float32)
    b1 = np.asarray(inputs["b1"], dtype=np.float32)
    b2 = np.asarray(inputs["b2"], dtype=np.float32)
    ln1_g = np.asarray(inputs["ln1_g"], dtype=np.float32)
    ln1_b = np.asarray(inputs["ln1_b"], dtype=np.float32)
    ln2_g = np.asarray(inputs["ln2_g"], dtype=np.float32)
    ln2_b = np.asarray(inputs["ln2_b"], dtype=np.float32)

    # fold the LN affine transforms into the weights/biases (host-side):
    # (z*g + b) @ W + c == z @ (diag(g) W) + (b @ W + c)
    wq_f = ln1_g[:, None] * wq
    bq_f = ln1_b @ wq + bq
    wkv_f = ln1_g[:, None] * wkv
    bkv_f = ln1_b @ wkv + bkv
    w1_f = ln2_g[:, None] * w1
    b1_f = ln2_b @ w1 + b1

    def bf(v):
        return np.ascontiguousarray(v.astype(ml_dtypes.bfloat16))

    def dm(v):  # [D] -> [P, 8] dim-major chunk layout
        return np.ascontiguousarray(v.reshape(8, P).T)

    consts_base = np.concatenate([
        (bq_f * ISD).reshape(8, P).T,     # bqs
        dm(bkv_f[0:D]),                   # bkvk
        b1_f.reshape(32, P).T,            # b1s
        dm(b2),                           # b2s
    ], axis=1)                            # [P, 56]; padb appended per core

    shared = {
        "wq": bf(wq_f),
        "wkv": bf(wkv_f),
        "w1": bf(w1_f),
        "w2": bf(w2),
        "bkvvb": np.ascontiguousarray(
            np.broadcast_to(bkv_f[D:2 * D], (P, D)).astype(np.float32)),
    }

    ki = np.arange(P)[:, None]   # key index within block (partition/row)
    qi = np.arange(P)[None, :]   # query index within block (free/col)
    tri = np.where(ki > qi, np.float32(EXPMASK), np.float32(KEEPVAL))
    keep = np.full((P, P), np.float32(KEEPVAL), dtype=np.float32)
    full = np.full((P, P), np.float32(EXPMASK), dtype=np.float32)

    in_maps = []
    for core in range(8):
        b, h = core // 2, core % 2
        perm = [2 * s + h for s in range(8)]
        xq = np.ascontiguousarray(
            x[b, 0:WINDOW * 2].reshape(16, P, D)[perm].reshape(WINDOW, D))
        xw = x[b, S - WINDOW:S]
        pad = kpm[b, S - WINDOW:S]
        m = dict(shared)
        m["xin"] = np.ascontiguousarray(
            np.concatenate([xq, xw], axis=0).astype(ml_dtypes.bfloat16))
        m["xinT"] = np.ascontiguousarray(xq.T)
        padb = (MASKVAL * pad.astype(np.float32)).reshape(8, P).T
        m["consts"] = np.ascontiguousarray(
            np.concatenate([consts_base, padb], axis=1))
        mE = tri if h == 0 else keep
        mO = full if h == 0 else tri
        m["masks"] = np.ascontiguousarray(
            np.concatenate([mE, mO], axis=1).astype(ml_dtypes.bfloat16))
        in_maps.append(m)
    return in_maps


def kernel(**inputs):
    from concourse.bass_utils import run_bass_kernel_spmd

    if "nc" not in _CACHE:
        _CACHE["nc"] = _build_program()
    nc = _CACHE["nc"]

    in_maps = _prep_inputs(inputs)
    trace = os.environ.get("KERNEL_TRACE", "0") == "1"
    res = run_bass_kernel_spmd(nc, in_maps, core_ids=list(range(8)),
                               trace=trace)
    if res.exec_time_ns is not None:
        print(f"HW exec time: {res.exec_time_ns} ns")
        _CACHE["exec_time_ns"] = res.exec_time_ns
    out = np.empty((B, S, D), dtype=np.float32)
    for core in range(8):
        b, h = core // 2, core % 2
        yT = res.results[core]["y"].T.reshape(8, P, D)
        dst = out[b, 0:WINDOW * 2].reshape(16, P, D)
        for s in range(8):
            dst[2 * s + h] = yT[s]
    return out


# revision 33
# speedup vs baseline: 1.4031x; 1.0034x over previous
"""Trainium2 Bass kernel for a custom transformer block.

Sharding: 8 cores = 4 batches x 2 interleaved query-chunk sets. Core (b, h)
owns query chunks {2s+h : s in 0..8} (128 tokens each) of batch b; the KV
window (last 1024 tokens) is recomputed on both cores of a batch pair. The
stride-2 interleave balances the causal-triangular attention work across the
pair and lets the score matmuls skip fully-masked key/query blocks: the
score strip for key chunk kc only covers queries from slot kc//2.

All matmul operands are bf16 (fp32 PSUM accumulation); LN stats, softmax
denominators, the MLP accumulator and the residual stay fp32. LN gains and
biases are folded into the weight matrices and projection biases host-side,
so the LN transpose evacuations are batched plain copies (8 transposes into
one PSUM bank, one ACT copy out). Padding masks ride the exp evacuation as
a per-key ACT bias (-80 * pad); the causal diagonal needs a 2D mask only on
the first 128-query block of each key strip, applied post-exp on the bf16
probabilities by the otherwise-idle GpSimd engine (min with exp(-80)/BIG).
Attention is software-pipelined: pair p's score strips (scalar-exp-bound)
interleave with pair p-1's AV matmuls and transposes to keep the PE dense.
Scores are computed transposed ([key, query]); row sums come from an extra
ones-column on V and the normalization is a per-partition vector multiply
after the PE transpose back to token-major.
"""
import sys
import os

if "/opt/trn_rl_repo" not in sys.path:
    sys.path.insert(0, "/opt/trn_rl_repo")

import numpy as np
import ml_dtypes

B, S, D = 4, 2048, 1024
N_HEAD = 16
D_HEAD = 64
WINDOW = 1024
D_FF = 4096
EPS = 1e-5
ISD = float(1.0 / np.sqrt(D))  # 1/32
MASKVAL = -80.0
EXPMASK = float(np.exp(-80.0))  # 1.8e-35: effectively zero, bf16-normal
KEEPVAL = 3e38
P = 128

# first live query slot for key chunk kc (strip start = 128*S_MIN[kc]);
# slot s holds query chunk 2s+h, live when kc <= 2s+h -> s >= ceil((kc-1)/2)
S_MIN = [kc // 2 for kc in range(8)]  # == ceil((kc-1)/2): [0,0,1,1,2,2,3,3]

_CACHE = {}


def _build_program():
    import concourse.bacc as bacc
    import concourse.mybir as mybir
    from concourse.tile import TileContext
    from concourse.masks import make_identity

    F32 = mybir.dt.float32
    BF16 = mybir.dt.bfloat16
    AF = mybir.ActivationFunctionType
    ALU = mybir.AluOpType
    AX = mybir.AxisListType

    nc = bacc.Bacc("TRN2", target_bir_lowering=False, debug=False,
                   num_devices=8)

    xin_d = nc.dram_tensor("xin", [2 * WINDOW, D], BF16,
                           kind="ExternalInput")
    wq_d = nc.dram_tensor("wq", [D, D], BF16, kind="ExternalInput")
    wkv_d = nc.dram_tensor("wkv", [D, 2 * D], BF16, kind="ExternalInput")
    w1_d = nc.dram_tensor("w1", [D, D_FF], BF16, kind="ExternalInput")
    w2_d = nc.dram_tensor("w2", [D_FF, D], BF16, kind="ExternalInput")
    # all small per-partition constants ride in one DMA:
    # [bqs 0:8 | bkvk 8:16 | b1s 16:48 | b2s 48:56 | padb 56:64]
    consts_d = nc.dram_tensor("consts", [P, 64], F32, kind="ExternalInput")
    bkvvb_d = nc.dram_tensor("bkvvb", [P, D], F32, kind="ExternalInput")
    masks_d = nc.dram_tensor("masks", [P, 2 * P], BF16, kind="ExternalInput")
    xinT_d = nc.dram_tensor("xinT", [D, WINDOW], F32, kind="ExternalInput")
    y_d = nc.dram_tensor("y", [D, WINDOW], F32, kind="ExternalOutput")

    with TileContext(nc) as tc:
        cpool = tc.alloc_tile_pool(name="const", bufs=1, side="left")
        identB = cpool.tile([P, P], BF16)
        make_identity(nc, identB[:])
        masks = cpool.tile([P, 2 * P], BF16)
        maskE = masks[:, 0:P]
        maskO = masks[:, P:2 * P]
        smallc = cpool.tile([P, 80], F32)
        bqs = smallc[:, 0:8]
        bkvk = smallc[:, 8:16]
        b1s = smallc[:, 16:48]
        b2s = smallc[:, 48:56]
        padb = smallc[:, 56:64]
        onesc = smallc[:, 64:80]
        nc.vector.memset(onesc, 1.0)

        def load_consts():  # deferred so the x DMAs win the queue
            nc.sync.dma_start(smallc[:, 0:64], consts_d[:])
            nc.sync.dma_start(masks[:], masks_d[:])

        # ---------------- Phase B/C: LN1 + QKV projections ------------------
        # z = (x - mu) * rstd token-major (LN gain/bias folded into weights);
        # 8 PE transposes batch into one PSUM bank, one ACT copy evacuates.
        # Window tiles (8-15) first so the V/K projections overlap the LN of
        # the query half, keeping the PE dense from the start.
        zTp = tc.alloc_tile_pool(name="zT", bufs=1, side="left")
        zqT = zTp.tile([P, 8, WINDOW], BF16)
        zwT = zTp.tile([P, 8, WINDOW], BF16)
        xz = tc.alloc_tile_pool(name="xz", bufs=3, side="left")
        psB = tc.alloc_tile_pool(name="psB", bufs=3, space="PSUM")

        def ln_stats(pool, xt, tag):
            """mean via DVE reduce, sum(x^2) via scalar Square+accumulator
            (runs in parallel); var = E[x^2] - mu^2."""
            st = pool.tile([P, 8], F32, tag="stats" + tag, name="st")
            junk = pool.tile([P, D], BF16, tag="junk" + tag, name="junk")
            musum, mu, sq = st[:, 0:1], st[:, 1:2], st[:, 2:3]
            mu2, veps, sdv, rstd = (st[:, 3:4], st[:, 4:5], st[:, 5:6],
                                    st[:, 6:7])
            nc.vector.reduce_sum(musum, xt, axis=AX.X)
            nc.scalar.activation(junk[:], xt, AF.Square, accum_out=sq)
            nc.vector.tensor_scalar_mul(mu, musum, 1.0 / D)
            nc.vector.tensor_tensor(mu2, mu, mu, op=ALU.mult)
            nc.vector.tensor_scalar(veps, sq, 1.0 / D, EPS,
                                    op0=ALU.mult, op1=ALU.add)
            nc.vector.tensor_tensor(veps, veps, mu2, op=ALU.subtract)
            nc.scalar.sqrt(sdv, veps)
            nc.vector.reciprocal(rstd, sdv)
            return mu, rstd

        def ln1_tile(t):
            xt = xz.tile([P, D], BF16, tag="x")
            nc.sync.dma_start(xt[:], xin_d[t * P:(t + 1) * P, :])
            mu, rstd = ln_stats(xz, xt[:], "1")
            z = xz.tile([P, D], BF16, tag="z")
            nc.vector.tensor_scalar(z[:], xt[:], mu, rstd,
                                    op0=ALU.subtract, op1=ALU.mult)
            batch = psB.tile([P, D], BF16, tag="tpB")
            for c in range(8):
                nc.tensor.transpose(batch[:, c * P:(c + 1) * P],
                                    z[:, c * P:(c + 1) * P], identB[:])
            dst = zqT if t < 8 else zwT
            col = (t % 8) * P
            # vector, not scalar: scalar already runs the Square pass, and
            # the V chains block on this copy during the window phase
            nc.vector.tensor_copy(dst[:, :, col:col + P],
                                  batch[:].rearrange("p (c n) -> p c n", n=P))

        for t in range(8, 16):
            ln1_tile(t)

        qkvp = tc.alloc_tile_pool(name="qkv", bufs=1, side="right")
        qT = qkvp.tile([P, 8, WINDOW], BF16)      # q/sqrt(D), dim-major
        kT = qkvp.tile([P, 8, WINDOW], BF16)      # k, dim-major
        V = qkvp.tile([P, 8, N_HEAD * 65], BF16)  # token-major + ones col

        wst = tc.alloc_tile_pool(name="wst", bufs=1, side="left")
        psC = tc.alloc_tile_pool(name="psC", bufs=3, space="PSUM")

        # V: activations stationary -> token-major, bias added via bcast tile
        bkvvb = wst.tile([P, D], F32, tag="bkvvb")
        nc.sync.dma_start(bkvvb[:], bkvvb_d[:])

        def wload(w_d, c0):  # [D, 512] weight block, dim-chunked
            wr = wst.tile([P, 8, 512], BF16, tag="wkres", bufs=4, name="wr")
            for kc in range(8):
                nc.sync.dma_start(
                    wr[:, kc, :], w_d[kc * P:(kc + 1) * P, c0:c0 + 512])
            return wr

        def v_chain(tt, vh, wvr):
            pp = psC.tile([P, 512], F32, tag="proj")
            for kc in range(8):
                nc.tensor.matmul(
                    pp[:], zwT[:, kc, tt * P:(tt + 1) * P],
                    wvr[:, kc, :],
                    start=(kc == 0), stop=(kc == 7))
            vdst = V[:, tt, :].rearrange("p (h n) -> p h n", n=65)[
                :, vh * 8:(vh + 1) * 8, 0:64]
            nc.vector.scalar_tensor_tensor(
                vdst, pp[:].rearrange("p (h n) -> p h n", n=64), 0.0,
                bkvvb[:, vh * 512:(vh + 1) * 512].rearrange(
                    "p (h n) -> p h n", n=64),
                op0=ALU.add, op1=ALU.add)

        def kq_half(wkr, wh, qh, dst, src, bias, scale):
            for co in range(wh * 4, wh * 4 + 4):
                pp = psC.tile([P, 512], F32, tag="proj")
                for kc in range(8):
                    nc.tensor.matmul(
                        pp[:], wkr[:, kc, (co % 4) * P:(co % 4 + 1) * P],
                        src[:, kc, qh * 512:(qh + 1) * 512],
                        start=(kc == 0), stop=(kc == 7))
                nc.scalar.activation(
                    dst[:, co, qh * 512:(qh + 1) * 512], pp[:],
                    AF.Identity, bias=bias[:, co:co + 1], scale=scale)

        # window LN first; V token-chunk chains interleave with it, then
        # K (window-only) and Q interleave with the query-half LN
        ln1_tile(8)
        ln1_tile(9)
        load_consts()
        wvr0 = wload(wkv_d, D)
        wvr1 = wload(wkv_d, D + 512)
        for tt in range(8):
            if tt + 10 < 16:
                ln1_tile(tt + 10)
            v_chain(tt, 0, wvr0)
            v_chain(tt, 1, wvr1)
        for tt in range(8):
            nc.scalar.copy(
                V[:, tt, :].rearrange("p (h n) -> p h n", n=65)[:, :, 64:65],
                onesc.rearrange("p (h n) -> p h n", n=1))
        wkr0 = wload(wkv_d, 0)
        wkr1 = wload(wkv_d, 512)
        ln1_tile(0)
        kq_half(wkr0, 0, 0, kT, zwT, bkvk, 1.0)
        ln1_tile(1)
        kq_half(wkr0, 0, 1, kT, zwT, bkvk, 1.0)
        ln1_tile(2)
        kq_half(wkr1, 1, 0, kT, zwT, bkvk, 1.0)
        ln1_tile(3)
        kq_half(wkr1, 1, 1, kT, zwT, bkvk, 1.0)
        wqr0 = wload(wq_d, 0)
        ln1_tile(4)
        kq_half(wqr0, 0, 0, qT, zqT, bqs, ISD)
        wqr1 = wload(wq_d, 512)
        ln1_tile(5)
        kq_half(wqr1, 1, 0, qT, zqT, bqs, ISD)
        ln1_tile(6)
        ln1_tile(7)
        kq_half(wqr0, 0, 1, qT, zqT, bqs, ISD)
        kq_half(wqr1, 1, 1, qT, zqT, bqs, ISD)

        # psC at 3 bufs leaves the score pool mostly on psB's banks, whose
        # tiles die before the last Q evacuations (less phase-handoff stall)
        psC.release()
        psB.release()
        wst.release()
        xz.release()
        zTp.release()

        # ---------------- Phase D: attention (software-pipelined) -----------
        attnp = tc.alloc_tile_pool(name="attn", bufs=1, side="left")
        attn = attnp.tile([P, 8, D], F32)          # normalized attn out
        rinva = attnp.tile([P, 8, N_HEAD], F32)    # per-head 1/rowsum
        # ping-pong exp(score) tiles: [pair-head, kc, query]; the columns
        # before each strip start are never written -> zero them once
        ptsA = attnp.tile([P, 2, 8, WINDOW], BF16)
        ptsB = attnp.tile([P, 2, 8, WINDOW], BF16)
        for pts in (ptsA, ptsB):
            for h2 in range(2):
                for kc in range(2, 8):
                    z0 = S_MIN[kc] * P
                    nc.vector.memset(pts[:, h2, kc, 0:z0], 0.0)

        sbD = tc.alloc_tile_pool(name="sbD", bufs=2, side="left")
        psDs = tc.alloc_tile_pool(name="psDs", bufs=2, space="PSUM")
        psDa = tc.alloc_tile_pool(name="psDa", bufs=2, space="PSUM")
        psDt = tc.alloc_tile_pool(name="psDt", bufs=2, space="PSUM")

        oa_tiles = {}

        def score_block(pts, pair, kc):
            z0 = S_MIN[kc] * P
            L = WINDOW - z0
            strips = {}
            for h in pair:
                strips[h] = psDs.tile([P, WINDOW], F32, tag="s", name="s")
            for c0, c1 in ((0, 512), (512, L)):
                for h in pair:
                    po, ch = (h % 2) * 64, h // 2
                    nc.tensor.matmul(
                        strips[h][:, c0:c1],
                        kT[po:po + 64, ch, kc * P:(kc + 1) * P],
                        qT[po:po + 64, ch, z0 + c0:z0 + c1],
                        start=True, stop=True)
            mk = maskE if kc % 2 == 0 else maskO
            for h in pair:
                nc.scalar.activation(
                    pts[:, h % 2, kc, z0:WINDOW], strips[h][:, 0:L],
                    AF.Exp, bias=padb[:, kc:kc + 1])
                # causal diagonal / dead block: min on the bf16 probs
                # (exp is monotone); all-SBUF 16-bit op -> 2x DVE rate
                nc.vector.tensor_tensor(
                    pts[:, h % 2, kc, z0:z0 + P],
                    pts[:, h % 2, kc, z0:z0 + P], mk, op=ALU.min)

        def av_chain(pts, h, qh, drain=False):
            if qh == 0:
                oa_tiles[h] = sbD.tile([65, WINDOW], BF16, tag="oa",
                                       name="oa")
            avp = psDa.tile([65, 512], F32, tag="av")
            for kc in range(8):
                nc.tensor.matmul(
                    avp[:], V[:, kc, h * 65:(h + 1) * 65],
                    pts[:, h % 2, kc, qh * 512:(qh + 1) * 512],
                    start=(kc == 0), stop=(kc == 7))
            dst = oa_tiles[h][:, qh * 512:(qh + 1) * 512]
            if drain:
                # scalar is idle once the last exp retires; the drain's
                # vector queue would stall the transposes otherwise
                nc.scalar.copy(dst, avp[:])
            else:
                nc.vector.tensor_copy(dst, avp[:])

        def trans_block(h, drain=False):
            oa = oa_tiles[h]
            # 66-wide groups keep each transpose output 4-byte aligned
            tpb = psDt.tile([P, 8, 66], BF16, tag="tp65")
            for t in range(8):
                nc.tensor.transpose(tpb[:, t, 0:65],
                                    oa[:, t * P:(t + 1) * P],
                                    identB[0:65, 0:65])
            for t in range(8):
                rinv = rinva[:, t, h:h + 1]
                nc.vector.reciprocal(rinv, tpb[:, t, 64:65])
                if drain:
                    # the drain's vector queue gates LN2; use idle scalar
                    nc.scalar.activation(
                        attn[:, t, h * 64:(h + 1) * 64], tpb[:, t, 0:64],
                        AF.Copy, scale=rinv)
                else:
                    nc.vector.tensor_scalar_mul(
                        attn[:, t, h * 64:(h + 1) * 64], tpb[:, t, 0:64],
                        rinv)

        for p in range(9):
            prev = []
            if p > 0:
                pv = ptsA if (p - 1) % 2 == 0 else ptsB
                for h in (2 * (p - 1), 2 * (p - 1) + 1):
                    dn = p == 8
                    prev.append(lambda h=h, pv=pv, dn=dn:
                                av_chain(pv, h, 0, dn))
                    prev.append(lambda h=h, pv=pv, dn=dn:
                                av_chain(pv, h, 1, dn))
                    prev.append(lambda h=h, dr=(p == 8 and h % 2 == 1):
                                trans_block(h, dr))
            if p < 8:
                pts = ptsA if p % 2 == 0 else ptsB
                pair = (2 * p, 2 * p + 1)
                for kc in range(8):
                    score_block(pts, pair, kc)
                    if kc < len(prev):
                        prev[kc]()
            else:
                for task in prev:
                    task()

        psDt.release()
        psDa.release()
        psDs.release()
        sbD.release()
        qkvp.release()

        # ---------------- Phase E: LN2 + transpose ---------------------------
        z2Tp = tc.alloc_tile_pool(name="z2T", bufs=1, side="right")
        z2T = z2Tp.tile([P, 8, WINDOW], BF16)
        # prefetch the first MLP weight block while LN2 runs
        wf1 = tc.alloc_tile_pool(name="wf1", bufs=1, side="right")
        w1r0 = wf1.tile([P, 8, 1024], BF16, tag="w1r", name="w1r0")
        for kc in range(8):
            nc.sync.dma_start(w1r0[:, kc, :], w1_d[kc * P:(kc + 1) * P, 0:1024])
        xz2 = tc.alloc_tile_pool(name="xz2", bufs=3, side="left")
        psE = tc.alloc_tile_pool(name="psE", bufs=3, space="PSUM")

        for t in range(8):
            at = attn[:, t, :]
            mu, rstd = ln_stats(xz2, at, "2")
            z = xz2.tile([P, D], BF16, tag="zE")
            nc.vector.tensor_scalar(z[:], at, mu, rstd,
                                    op0=ALU.subtract, op1=ALU.mult)
            batch = psE.tile([P, D], BF16, tag="tpE")
            for c in range(8):
                nc.tensor.transpose(batch[:, c * P:(c + 1) * P],
                                    z[:, c * P:(c + 1) * P], identB[:])
            nc.scalar.copy(z2T[:, :, t * P:(t + 1) * P],
                           batch[:].rearrange("p (c n) -> p c n", n=P))

        psE.release()
        xz2.release()
        attnp.release()

        # ---------------- Phase F: MLP + residual ----------------------------
        h2p = tc.alloc_tile_pool(name="h2acc", bufs=1, side="left")
        h2acc = h2p.tile([P, 8, WINDOW], F32)
        xinTp = tc.alloc_tile_pool(name="xinT", bufs=1, side="left")
        xinT = xinTp.tile([P, 8, WINDOW], F32)
        nc.sync.dma_start(xinT[:], xinT_d.rearrange("(c p) n -> p c n", p=P))
        wf2 = tc.alloc_tile_pool(name="wf2", bufs=1, side="right")
        h1p = tc.alloc_tile_pool(name="h1p", bufs=1, side="left")
        tailp = tc.alloc_tile_pool(name="tail", bufs=3, side="left")
        psF1 = tc.alloc_tile_pool(name="psF1", bufs=4, space="PSUM")
        psF2 = tc.alloc_tile_pool(name="psF2", bufs=4, space="PSUM")

        for sc in range(4):
            if sc == 0:
                w1r = w1r0
            else:
                w1r = wf1.tile([P, 8, 1024], BF16, tag="w1r")
                for kc in range(8):
                    nc.sync.dma_start(
                        w1r[:, kc, :],
                        w1_d[kc * P:(kc + 1) * P, sc * 1024:(sc + 1) * 1024])
            h1 = h1p.tile([P, 8, WINDOW], BF16, tag="h1")
            # qh-outer: the first half only needs LN2 tiles 0-3, so the
            # MLP starts while the second half of LN2 is still running
            for qh in range(2):
                for ft in range(8):
                    hp = psF1.tile([P, 512], F32, tag="h1ps")
                    for kc in range(8):
                        nc.tensor.matmul(
                            hp[:], w1r[:, kc, ft * P:(ft + 1) * P],
                            z2T[:, kc, qh * 512:(qh + 1) * 512],
                            start=(kc == 0), stop=(kc == 7))
                    nc.scalar.activation(
                        h1[:, ft, qh * 512:(qh + 1) * 512], hp[:], AF.Silu,
                        bias=b1s[:, sc * 8 + ft:sc * 8 + ft + 1], scale=1.0)
            w2r = wf2.tile([P, 8, 1024], BF16, tag="w2r")
            for kc in range(8):
                nc.sync.dma_start(
                    w2r[:, kc, :],
                    w2_d[(sc * 8 + kc) * P:(sc * 8 + kc + 1) * P, :])
            for co in range(8):
                for qh in range(2):
                    hp2 = psF2.tile([P, 512], F32, tag="h2ps")
                    for kc in range(8):
                        nc.tensor.matmul(
                            hp2[:], w2r[:, kc, co * P:(co + 1) * P],
                            h1[:, kc, qh * 512:(qh + 1) * 512],
                            start=(kc == 0), stop=(kc == 7))
                    dstp = h2acc[:, co, qh * 512:(qh + 1) * 512]
                    if sc == 0:
                        nc.vector.tensor_copy(dstp, hp2[:])
                    elif sc < 3:
                        nc.vector.tensor_tensor(dstp, hp2[:], dstp,
                                                op=ALU.add)
                    else:
                        nc.vector.scalar_tensor_tensor(
                            dstp, hp2[:], b2s[:, co:co + 1], dstp,
                            op0=ALU.add, op1=ALU.add)
                if sc == 3:
                    # residual add + store as soon as this dim chunk is done
                    y = tailp.tile([P, WINDOW], F32, tag="y")
                    nc.vector.tensor_tensor(y[:], h2acc[:, co, :],
                                            xinT[:, co, :], op=ALU.add)
                    nc.sync.dma_start(y_d[co * P:(co + 1) * P, :], y[:])

        psF2.release()
        psF1.release()
        tailp.release()
        h1p.release()
        wf2.release()
        wf1.release()
        z2Tp.release()
        xinTp.release()
        h2p.release()
        cpool.release()

    nc.compile()
    return nc


def _prep_inputs(inputs):
    x = np.ascontiguousarray(np.asarray(inputs["x"], dtype=np.float32))
    kpm = np.asarray(inputs["key_pad_mask"]).astype(bool)
    wq = np.asarray(inputs["wq"], dtype=np.float32)
    wkv = np.asarray(inputs["wkv"], dtype=np.float32)
    w1 = np.asarray(inputs["w1"], dtype=np.float32)
    w2 = np.asarray(inputs["w2"], dtype=np.float32)
    bq = np.asarray(inputs["bq"], dtype=np.float32)
    bkv = np.asarray(inputs["bkv"], dtype=np.float32)
    b1 = np.asarray(inputs["b1"], dtype=np.float32)
    b2 = np.asarray(inputs["b2"], dtype=np.float32)
    ln1_g = np.asarray(inputs["ln1_g"], dtype=np.float32)
    ln1_b = np.asarray(inputs["ln1_b"], dtype=np.float32)
    ln2_g = np.asarray(inputs["ln2_g"], dtype=np.float32)
    ln2_b = np.asarray(inputs["ln2_b"], dtype=np.float32)

    # fold the LN affine transforms into the weights/biases (host-side):
    # (z*g + b) @ W + c == z @ (diag(g) W) + (b @ W + c)
    wq_f = ln1_g[:, None] * wq
    bq_f = ln1_b @ wq + bq
    wkv_f = ln1_g[:, None] * wkv
    bkv_f = ln1_b @ wkv + bkv
    w1_f = ln2_g[:, None] * w1
    b1_f = ln2_b @ w1 + b1

    def bf(v):
        return np.ascontiguousarray(v.astype(ml_dtypes.bfloat16))

    def dm(v):  # [D] -> [P, 8] dim-major chunk layout
        return np.ascontiguousarray(v.reshape(8, P).T)

    consts_base = np.concatenate([
        (bq_f * ISD).reshape(8, P).T,     # bqs
        dm(bkv_f[0:D]),                   # bkvk
        b1_f.reshape(32, P).T,            # b1s
        dm(b2),                           # b2s
    ], axis=1)                            # [P, 56]; padb appended per core

    shared = {
        "wq": bf(wq_f),
        "wkv": bf(wkv_f),
        "w1": bf(w1_f),
        "w2": bf(w2),
        "bkvvb": np.ascontiguousarray(
            np.broadcast_to(bkv_f[D:2 * D], (P, D)).astype(np.float32)),
    }

    ki = np.arange(P)[:, None]   # key index within block (partition/row)
    qi = np.arange(P)[None, :]   # query index within block (free/col)
    tri = np.where(ki > qi, np.float32(EXPMASK), np.float32(KEEPVAL))
    keep = np.full((P, P), np.float32(KEEPVAL), dtype=np.float32)
    full = np.full((P, P), np.float32(EXPMASK), dtype=np.float32)

    in_maps = []
    for core in range(8):
        b, h = core // 2, core % 2
        perm = [2 * s + h for s in range(8)]
        xq = np.ascontiguousarray(
            x[b, 0:WINDOW * 2].reshape(16, P, D)[perm].reshape(WINDOW, D))
        xw = x[b, S - WINDOW:S]
        pad = kpm[b, S - WINDOW:S]
        m = dict(shared)
        m["xin"] = np.ascontiguousarray(
            np.concatenate([xq, xw], axis=0).astype(ml_dtypes.bfloat16))
        m["xinT"] = np.ascontiguousarray(xq.T)
        padb = (MASKVAL * pad.astype(np.float32)).reshape(8, P).T
        m["consts"] = np.ascontiguousarray(
            np.concatenate([consts_base, padb], axis=1))
        mE = tri if h == 0 else keep
        mO = full if h == 0 else tri
        m["masks"] = np.ascontiguousarray(
            np.concatenate([mE, mO], axis=1).astype(ml_dtypes.bfloat16))
        in_maps.append(m)
    return in_maps


def kernel(**inputs):
    from concourse.bass_utils import run_bass_kernel_spmd

    if "nc" not in _CACHE:
        _CACHE["nc"] = _build_program()
    nc = _CACHE["nc"]

    in_maps = _prep_inputs(inputs)
    trace = os.environ.get("KERNEL_TRACE", "0") == "1"
    res = run_bass_kernel_spmd(nc, in_maps, core_ids=list(range(8)),
                               trace=trace)
    if res.exec_time_ns is not None:
        print(f"HW exec time: {res.exec_time_ns} ns")
        _CACHE["exec_time_ns"] = res.exec_time_ns
    out = np.empty((B, S, D), dtype=np.float32)
    for core in range(8):
        b, h = core // 2, core % 2
        yT = res.results[core]["y"].T.reshape(8, P, D)
        dst = out[b, 0:WINDOW * 2].reshape(16, P, D)
        for s in range(8):
            dst[2 * s + h] = yT[s]
    return out
